# revision 1
# baseline (speedup 1.0000x reference)
"""GAT (2-layer, 4-head + 1-head) + global mean pool + linear head on 8 TRN2 cores.

Strategy (per sharding hint): nodes (and their incident edges, partitioned by
dst) are sharded across 8 cores; small weights replicated.

Launch 1 (dense): each core computes T1 = [as|ad|h1] for its own 1/8 of the
  nodes (block-aligned slice of x^T @ Waug). Host stitches the 8 slices into
  the full T1 table (int16-safe A/B halves).
Launch 2 (layer-1 edge phase): per-window (128 dst nodes) edge attention:
  dma_gather of T1[src] rows, indicator-matrix matmuls for per-dst softmax
  denominators and aggregation -> dense2 (h2 + layer-2 logits). Outputs
  per-core T2 rows (as2|h2) and per-edge al_dst2. Host stitches full T2
  (bf16) table.
Launch 3 (layer-2 edge phase): gather T2[src] (256B bf16 rows), same
  indicator aggregation -> global mean pool partials -> partial [64,2]
  logits. Host sums the 8 partials + bl.

Host work is limited to sharding/layout prep (edge sort/partition, index
lists, transposes/padding/dtype casts, per-graph node counts) and unshard
(concat/stitch of device-computed T1/T2 slices, sum of partial logits).
"""

import contextlib
import hashlib
import os
import numpy as np
import ml_dtypes

import concourse.bass as bass
import concourse.mybir as mybir
import concourse.tile as tile
from concourse import bacc
from concourse import bass_utils
from concourse.masks import make_identity

bf16 = ml_dtypes.bfloat16
F32 = mybir.dt.float32
BF16 = mybir.dt.bfloat16
I16 = mybir.dt.int16
AF = mybir.ActivationFunctionType
ALU = mybir.AluOpType

# ---- problem constants ----
N_NODES = 50000
N_GRAPHS = 64
F_IN = 500
F_IN_PAD = 512
H1 = 256          # heads*hid layer 1
HEADS = 4
HID = 64
NEG_SLOPE = 0.2
NCORES = 8
OWN = N_NODES // NCORES          # 6250
P = 128
NODES_PAD = 50048                # 391*128
NBLK = NODES_PAD // P            # 391
WINDOWS = (OWN + P - 1) // P     # 49
LAST_ROWS = OWN - (WINDOWS - 1) * P   # 106
OWNPAD = WINDOWS * P             # 6272
NBLK_CORE = 50                   # xT blocks per core in dense launch
TAB_HALF = 195 * P               # 24960: block-aligned int16-safe table split
TAB_A = TAB_HALF                 # rows in table A
TAB_B = NODES_PAD - TAB_HALF     # 25088 rows in table B (< 32767)
T1_COLS = 384                    # bf16 row: [as(4) | ad(4) | h1(256) | junk(120)]
T2_COLS = 128                    # bf16 row: [as2(1) | h2(64) | junk(63)]
EPS = 1e-16

TRACE = bool(int(os.environ.get("KERNEL_TRACE", "0")))
SP_A = bool(int(os.environ.get("KERNEL_SP_A", "0")))
SP_B = bool(int(os.environ.get("KERNEL_SP_B", "0")))
LAST_TIMES = {}

_CACHE = {}


# ======================================================================
# host preprocessing
# ======================================================================

def _wrap_idx(idx, L):
    pad = np.zeros(L, np.int32)
    pad[: len(idx)] = idx
    return pad.reshape(L // 16, 16).T.astype(np.int16)  # [16, L/16]


def _prep(edge_index, batch):
    src = np.concatenate([edge_index[0], np.arange(N_NODES, dtype=np.int64)])
    dst = np.concatenate([edge_index[1], np.arange(N_NODES, dtype=np.int64)])
    src = src.astype(np.int32)
    dst = dst.astype(np.int32)

    coreinfo = []
    nA = np.zeros((NCORES, WINDOWS), np.int64)
    nB = np.zeros((NCORES, WINDOWS), np.int64)
    for k in range(NCORES):
        m = (dst >= k * OWN) & (dst < (k + 1) * OWN)
        s = src[m]
        d = dst[m] - k * OWN
        w = d >> 7
        order = np.lexsort((s, w))
        s, d, w = s[order], d[order], w[order]
        isA = s < TAB_HALF
        wins = []
        wstart = np.searchsorted(w, np.arange(WINDOWS + 1))
        for wi in range(WINDOWS):
            sl = slice(wstart[wi], wstart[wi + 1])
            sw, dw, aw = s[sl], d[sl], isA[sl]
            wins.append((sw[aw], dw[aw] - wi * P, sw[~aw] - TAB_HALF,
                         dw[~aw] - wi * P))
            nA[k, wi] = int(aw.sum())
            nB[k, wi] = int((~aw).sum())
        coreinfo.append(wins)

    mA = [max(1, int(np.ceil(nA[:, w].max() / P))) for w in range(WINDOWS)]
    mB = [max(1, int(np.ceil(nB[:, w].max() / P))) for w in range(WINDOWS)]
    mW = [a + b for a, b in zip(mA, mB)]
    dims = dict(mA=mA, mB=mB, mW=mW,
                sumA=sum(mA) * P, sumB=sum(mB) * P,
                sumM=sum(mW), sumE=sum(mW) * P, mmax=max(mW))

    per_core = []
    for k in range(NCORES):
        idxA = np.zeros((16, dims["sumA"] // 16), np.int16)
        idxB = np.zeros((16, dims["sumB"] // 16), np.int16)
        dstcol = np.full((dims["sumM"], P), -1.0, bf16)
        maskc = np.zeros((dims["sumM"], P), bf16)
        cA = cB = cM = 0
        for w in range(WINDOWS):
            sA, dA, sB, dB = coreinfo[k][w]
            LA, LB = mA[w] * P, mB[w] * P
            idxA[:, cA // 16:(cA + LA) // 16] = _wrap_idx(sA, LA)
            idxB[:, cB // 16:(cB + LB) // 16] = _wrap_idx(sB, LB)
            dv = np.full(LA + LB, -1.0, np.float32)
            dv[: len(dA)] = dA
            dv[LA: LA + len(dB)] = dB
            mv = np.zeros(LA + LB, np.float32)
            mv[: len(dA)] = 1.0
            mv[LA: LA + len(dB)] = 1.0
            dstcol[cM:cM + mW[w]] = dv.reshape(mW[w], P).astype(bf16)
            maskc[cM:cM + mW[w]] = mv.reshape(mW[w], P).astype(bf16)
            cA += LA
            cB += LB
            cM += mW[w]

        bv = np.full((OWNPAD,), -1.0, np.float32)
        bv[:OWN] = batch[k * OWN:(k + 1) * OWN].astype(np.float32)
        per_core.append(dict(
            idxA=idxA, idxB=idxB, dstcol=dstcol,
            dstT=np.ascontiguousarray(dstcol.T),
            mskT=np.ascontiguousarray(maskc.T),
            bvT=np.ascontiguousarray(
                bv.astype(bf16).reshape(WINDOWS, P).T)))
    return dims, per_core


def _prep_weights(x, W1, a_src1, a_dst1, W2, a_src2, a_dst2):
    xT = np.zeros((F_IN_PAD, NODES_PAD), bf16)
    xT[:F_IN, :N_NODES] = x.T.astype(bf16)

    Asrc = np.zeros((H1, HEADS), np.float32)
    Adst = np.zeros((H1, HEADS), np.float32)
    for h in range(HEADS):
        Asrc[h * HID:(h + 1) * HID, h] = a_src1[h]
        Adst[h * HID:(h + 1) * HID, h] = a_dst1[h]
    Waug = np.zeros((F_IN_PAD, 8 + H1), np.float32)
    Waug[:F_IN, 0:4] = W1 @ Asrc
    Waug[:F_IN, 4:8] = W1 @ Adst
    Waug[:F_IN, 8:] = W1
    Waug = Waug.astype(bf16)

    W2aug = np.zeros((H1, HID + 2), np.float32)
    W2aug[:, :HID] = W2
    W2aug[:, HID] = W2 @ a_src2[0]
    W2aug[:, HID + 1] = W2 @ a_dst2[0]
    W2aug = W2aug.astype(bf16)
    return xT, Waug, W2aug


# ======================================================================
# launch 1: dense1 (sharded across cores)
# ======================================================================

def build_dense():
    nc = bacc.Bacc("TRN2", target_bir_lowering=False, debug=False)

    NC = NBLK_CORE * P  # 6400 nodes per core (block-aligned, overlapping)
    xTc_d = nc.dram_tensor("xTc", [F_IN_PAD, NC], BF16, kind="ExternalInput")
    Waug_d = nc.dram_tensor("Waug", [F_IN_PAD, 264], BF16, kind="ExternalInput")
    T1k_d = nc.dram_tensor("T1k", [NC, 264], BF16, kind="ExternalOutput")

    with tile.TileContext(nc) as tc:
        ctx = contextlib.ExitStack()
        with ctx:
            const = ctx.enter_context(tc.tile_pool(name="const", bufs=1))
            waug_t = const.tile([P, 4, 264], BF16)
            nc.sync.dma_start(waug_t[:], Waug_d[:].rearrange("(ko p) c -> p ko c", p=P))

            CH = 10  # node blocks per xT chunk
            with tc.tile_pool(name="dense", bufs=3) as dpool, \
                 tc.tile_pool(name="dpsum", bufs=4, space="PSUM") as dps:
                for c0 in range(0, NBLK_CORE, CH):
                    nchunk = min(CH, NBLK_CORE - c0) * P
                    xt_t = dpool.tile([P, 4, CH * P], BF16, tag="xt")
                    nc.sync.dma_start(
                        xt_t[:, :, :nchunk],
                        xTc_d[:].rearrange("(ko p) n -> p ko n", p=P)[
                            :, :, c0 * P: c0 * P + nchunk],
                    )
                    for b in range(nchunk // P):
                        ps = dps.tile([P, 264], F32, tag="dps")
                        for ko in range(4):
                            nc.tensor.matmul(
                                ps[:],
                                lhsT=xt_t[:, ko, b * P:(b + 1) * P],
                                rhs=waug_t[:, ko, :],
                                start=(ko == 0),
                                stop=(ko == 3),
                            )
                        t1_t = dpool.tile([P, 264], BF16, tag="t1")
                        nc.scalar.copy(t1_t[:], ps[:])
                        nb = c0 + b
                        nc.sync.dma_start(
                            T1k_d[nb * P:(nb + 1) * P, :], t1_t[:])

    nc.compile()
    return nc


# ======================================================================
# launch 2: layer-1 edge phase + dense2
# ======================================================================

def build_phase_a(dims):
    mA, mB, mW = dims["mA"], dims["mB"], dims["mW"]
    mmax = dims["mmax"]
    nc = bacc.Bacc("TRN2", target_bir_lowering=False, debug=False)

    T1a_d = nc.dram_tensor("T1a", [TAB_A, T1_COLS], BF16, kind="ExternalInput")
    T1b_d = nc.dram_tensor("T1b", [TAB_B, T1_COLS], BF16, kind="ExternalInput")
    W2aug_d = nc.dram_tensor("W2aug", [H1, 66], BF16, kind="ExternalInput")
    idxA_d = nc.dram_tensor("idxA", [16, dims["sumA"] // 16], I16, kind="ExternalInput")
    idxB_d = nc.dram_tensor("idxB", [16, dims["sumB"] // 16], I16, kind="ExternalInput")
    dstcol_d = nc.dram_tensor("dstcol", [dims["sumM"], P], BF16, kind="ExternalInput")
    dstT_d = nc.dram_tensor("dstT", [P, dims["sumM"]], BF16, kind="ExternalInput")
    mskT_d = nc.dram_tensor("mskT", [P, dims["sumM"]], BF16, kind="ExternalInput")
    ownblk_d = nc.dram_tensor("ownblk", [OWNPAD, 8], BF16, kind="ExternalInput")
    iotaF_d = nc.dram_tensor("iotaF", [1, P], BF16, kind="ExternalInput")
    iotaC_d = nc.dram_tensor("iotaC", [P, 1], BF16, kind="ExternalInput")
    b1_d = nc.dram_tensor("b1r", [1, H1], F32, kind="ExternalInput")

    T2own_d = nc.dram_tensor("T2own", [OWNPAD, 65], F32, kind="ExternalOutput")
    ad2_d = nc.dram_tensor("ad2", [dims["sumE"]], F32, kind="ExternalOutput")

    with tile.TileContext(nc) as tc:
        ctx = contextlib.ExitStack()
        with ctx:
            const = ctx.enter_context(tc.tile_pool(name="const", bufs=1))
            # upfront bulk metadata — index loads FIRST (gathers wait on them)
            iaAll = const.tile([P, dims["sumA"] // 16], I16)
            nc.sync.dma_start(
                iaAll[:], idxA_d[None, :, :].to_broadcast(
                    [8, 16, dims["sumA"] // 16]))
            ibAll = const.tile([P, dims["sumB"] // 16], I16)
            nc.sync.dma_start(
                ibAll[:], idxB_d[None, :, :].to_broadcast(
                    [8, 16, dims["sumB"] // 16]))
            dstT = const.tile([P, dims["sumM"]], BF16)
            nc.sync.dma_start(dstT[:], dstT_d[:])
            mskT = const.tile([P, dims["sumM"]], BF16)
            nc.sync.dma_start(mskT[:], mskT_d[:])
            w2aug_t = const.tile([P, 2, 66], BF16)
            nc.sync.dma_start(w2aug_t[:], W2aug_d[:].rearrange("(ko p) c -> p ko c", p=P))
            iotaF_t = const.tile([P, P], BF16)
            nc.sync.dma_start(iotaF_t[:], iotaF_d[:].to_broadcast([P, P]))
            iotaC_t = const.tile([P, 1], BF16)
            nc.sync.dma_start(iotaC_t[:], iotaC_d[:])
            b1_t = const.tile([P, H1], F32)
            nc.sync.dma_start(b1_t[:], b1_d[:].to_broadcast([P, H1]))
            ident_t = const.tile([P, P], F32)
            make_identity(nc, ident_t[:])
            ones_t = const.tile([1, P], BF16)
            nc.vector.memset(ones_t[:], 1.0)
            # own-node [as|ad] cache (host-sliced from stitched T1)
            ocp = const.tile([P, WINDOWS, 8], BF16)
            nc.sync.dma_start(
                ocp[:], ownblk_d[:].rearrange("(w p) c -> p w c", p=P))

            wpool = ctx.enter_context(tc.tile_pool(name="win", bufs=2))
            spool = ctx.enter_context(tc.tile_pool(name="small", bufs=2))
            ps_dr = ctx.enter_context(tc.tile_pool(name="psdr", bufs=2, space="PSUM"))
            ps_ad1 = ctx.enter_context(tc.tile_pool(name="psad1", bufs=1, space="PSUM"))
            ps_agg = ctx.enter_context(tc.tile_pool(name="psagg", bufs=2, space="PSUM"))
            ps_z1t = ctx.enter_context(tc.tile_pool(name="psz1t", bufs=1, space="PSUM"))
            ps_h2 = ctx.enter_context(tc.tile_pool(name="psh2", bufs=1, space="PSUM"))
            ps_ad2 = ctx.enter_context(tc.tile_pool(name="psad2", bufs=1, space="PSUM"))

            cA = cB = cM = cE = 0
            for w in range(WINDOWS):
                ma, mb, m = mA[w], mB[w], mW[w]
                Ew = m * P
                rows = LAST_ROWS if w == WINDOWS - 1 else P

                # --- gather T1[src] rows for this window's edges ---
                v_t = wpool.tile([P, mmax, T1_COLS], BF16, tag="v")
                nc.gpsimd.dma_gather(
                    out_ap=v_t[:, 0:ma, :], in_ap=T1a_d[:],
                    idxs_ap=iaAll[:, cA // 16:(cA + ma * P) // 16],
                    num_idxs=ma * P, num_idxs_reg=ma * P, elem_size=T1_COLS,
                    single_packet=SP_A)
                nc.gpsimd.dma_gather(
                    out_ap=v_t[:, ma:m, :], in_ap=T1b_d[:],
                    idxs_ap=ibAll[:, cB // 16:(cB + mb * P) // 16],
                    num_idxs=mb * P, num_idxs_reg=mb * P, elem_size=T1_COLS,
                    single_packet=SP_A)
                dcol_t = dstT[:, cM:cM + m]
                msk_t = mskT[:, cM:cM + m]
                drow_t = wpool.tile([1, mmax * P], BF16, tag="drow")
                nc.sync.dma_start(
                    drow_t[:, :Ew],
                    dstcol_d[cM:cM + m, :].rearrange("j p -> (j p)")[None, :])

                # --- S (edge-major indicator) ---
                s_t = wpool.tile([P, mmax, P], BF16, tag="s")
                nc.vector.tensor_tensor(
                    s_t[:, :m, :],
                    dcol_t[:, :, None].to_broadcast([P, m, P]),
                    iotaF_t[:, None, :].to_broadcast([P, m, P]),
                    ALU.is_equal)
                # --- S_T (dst-major indicator) via PE row-broadcast ---
                drb_t = wpool.tile([P, mmax * P], BF16, tag="drb")
                for c0 in range(0, Ew, 512):
                    cw = min(512, Ew - c0)
                    psd = ps_dr.tile([P, 512], F32, tag="psdr")
                    nc.tensor.matmul(
                        psd[:, :cw], lhsT=ones_t[:], rhs=drow_t[:, c0:c0 + cw],
                        start=True, stop=True)
                    nc.scalar.copy(drb_t[:, c0:c0 + cw], psd[:, :cw])
                str_t = wpool.tile([P, mmax * P], BF16, tag="str")
                nc.vector.tensor_tensor(
                    str_t[:, :Ew],
                    iotaC_t[:].to_broadcast([P, Ew]),
                    drb_t[:, :Ew],
                    ALU.is_equal)

                # --- ad1 per edge ---
                pad1 = ps_ad1.tile([P, 4 * mmax], F32, tag="psad1")
                for j in range(m):
                    nc.tensor.matmul(
                        pad1[:, j * 4:(j + 1) * 4],
                        lhsT=str_t[:, j * P:(j + 1) * P],
                        rhs=ocp[:, w, 4:8],
                        start=True, stop=True)
                # --- ex = exp(lrelu(as + ad)) * mask ---
                zf = spool.tile([P, mmax, 4], F32, tag="zf")
                nc.vector.tensor_tensor(
                    zf[:, :m, :], v_t[:, :m, 0:4],
                    pad1[:].rearrange("p (j c) -> p j c", c=4)[:, :m, :],
                    ALU.add)
                zt = spool.tile([P, mmax, 4], F32, tag="zt")
                nc.vector.tensor_scalar_mul(zt[:, :m, :], zf[:, :m, :], NEG_SLOPE)
                nc.vector.tensor_tensor(zt[:, :m, :], zt[:, :m, :], zf[:, :m, :],
                                        ALU.max)
                ex_t = spool.tile([P, mmax, 4], BF16, tag="ex")
                nc.scalar.activation(ex_t[:, :m, :], zt[:, :m, :], AF.Exp)
                nc.vector.tensor_tensor(
                    ex_t[:, :m, :], ex_t[:, :m, :],
                    msk_t[:, :, None].to_broadcast([P, m, 4]), ALU.mult)
                # --- Vw = [h*ex | ex] ---
                vw_t = wpool.tile([P, mmax, 260], BF16, tag="vw")
                nc.vector.tensor_tensor(
                    vw_t[:, :m, 0:256].rearrange("p m (h c) -> p m h c", h=HEADS),
                    v_t[:, :m, 8:264].rearrange("p m (h c) -> p m h c", h=HEADS),
                    ex_t[:, :m, :, None].to_broadcast([P, m, HEADS, HID]),
                    ALU.mult)
                nc.vector.tensor_copy(vw_t[:, :m, 256:260], ex_t[:, :m, :])

                # --- aggregate ---
                pagg = ps_agg.tile([P, 260], F32, tag="psagg")
                for j in range(m):
                    nc.tensor.matmul(
                        pagg[:], lhsT=s_t[:, j, :], rhs=vw_t[:, j, :],
                        start=(j == 0), stop=(j == m - 1))
                # --- out1 = agg / s + b1 ; z1 = relu ---
                sden = spool.tile([P, 4], F32, tag="sden")
                nc.vector.tensor_scalar_add(sden[:], pagg[:, 256:260], EPS)
                nc.vector.reciprocal(sden[:], sden[:])
                z1 = spool.tile([P, H1], F32, tag="z1")
                nc.vector.tensor_tensor(
                    z1[:].rearrange("p (h c) -> p h c", h=HEADS),
                    pagg[:, 0:256].rearrange("p (h c) -> p h c", h=HEADS),
                    sden[:, :, None].to_broadcast([P, HEADS, HID]),
                    ALU.mult)
                nc.vector.tensor_add(z1[:], z1[:], b1_t[:])
                nc.scalar.activation(z1[:], z1[:], AF.Relu)

                # --- dense 2: h2aug = z1 @ W2aug ---
                z1t = spool.tile([P, 2, P], BF16, tag="z1t")
                for hh in range(2):
                    pzt = ps_z1t.tile([P, P], F32, tag="psz1t")
                    nc.tensor.transpose(
                        pzt[:], z1[:, hh * P:(hh + 1) * P], ident_t[:])
                    nc.scalar.copy(z1t[:, hh, :], pzt[:])
                ph2 = ps_h2.tile([P, 66], F32, tag="psh2")
                for hh in range(2):
                    nc.tensor.matmul(
                        ph2[:], lhsT=z1t[:, hh, :], rhs=w2aug_t[:, hh, :],
                        start=(hh == 0), stop=(hh == 1))
                t2_t = spool.tile([P, 65], F32, tag="t2")
                nc.scalar.copy(t2_t[:, 0:1], ph2[:, 64:65])
                nc.scalar.copy(t2_t[:, 1:65], ph2[:, 0:64])
                nc.sync.dma_start(
                    T2own_d[w * P: w * P + rows, :], t2_t[:rows, :])

                # --- ad2 per edge (for phase B) ---
                ald2 = spool.tile([P, 1], BF16, tag="ald2")
                nc.scalar.copy(ald2[:], ph2[:, 65:66])
                pad2 = ps_ad2.tile([P, mmax], F32, tag="psad2")
                for j in range(m):
                    nc.tensor.matmul(
                        pad2[:, j:j + 1],
                        lhsT=str_t[:, j * P:(j + 1) * P],
                        rhs=ald2[:], start=True, stop=True)
                ad2s = spool.tile([P, mmax], F32, tag="ad2s")
                nc.vector.tensor_copy(ad2s[:, :m], pad2[:, :m])
                nc.sync.dma_start(
                    ad2_d[cE:cE + Ew].rearrange("(j p) -> p j", p=P),
                    ad2s[:, :m])

                cA += ma * P
                cB += mb * P
                cM += m
                cE += Ew

    nc.compile()
    return nc


# ======================================================================
# launch 3: layer-2 edge phase + pool + logits
# ======================================================================

def build_phase_b(dims):
    mA, mB, mW = dims["mA"], dims["mB"], dims["mW"]
    mmax = dims["mmax"]
    nc = bacc.Bacc("TRN2", target_bir_lowering=False, debug=False)

    T2A_d = nc.dram_tensor("T2A", [TAB_A, T2_COLS], BF16, kind="ExternalInput")
    T2B_d = nc.dram_tensor("T2B", [TAB_B, T2_COLS], BF16, kind="ExternalInput")
    idxA_d = nc.dram_tensor("idxA", [16, dims["sumA"] // 16], I16, kind="ExternalInput")
    idxB_d = nc.dram_tensor("idxB", [16, dims["sumB"] // 16], I16, kind="ExternalInput")
    dstT_d = nc.dram_tensor("dstT", [P, dims["sumM"]], BF16, kind="ExternalInput")
    mskT_d = nc.dram_tensor("mskT", [P, dims["sumM"]], BF16, kind="ExternalInput")
    ad2T_d = nc.dram_tensor("ad2T", [P, dims["sumM"]], F32, kind="ExternalInput")
    iotaF_d = nc.dram_tensor("iotaF", [1, P], BF16, kind="ExternalInput")
    giota_d = nc.dram_tensor("giota", [1, N_GRAPHS], BF16, kind="ExternalInput")
    bvT_d = nc.dram_tensor("bvT", [P, WINDOWS], BF16, kind="ExternalInput")
    b2_d = nc.dram_tensor("b2r", [1, HID], F32, kind="ExternalInput")
    cnt_d = nc.dram_tensor("cnt", [N_GRAPHS, 1], F32, kind="ExternalInput")
    Wl_d = nc.dram_tensor("Wl", [HID, 2], F32, kind="ExternalInput")

    out_d = nc.dram_tensor("partial", [N_GRAPHS, 2], F32, kind="ExternalOutput")

    with tile.TileContext(nc) as tc:
        ctx = contextlib.ExitStack()
        with ctx:
            const = ctx.enter_context(tc.tile_pool(name="const", bufs=1))
            # upfront bulk metadata — index loads FIRST (gathers wait on them)
            iaAll = const.tile([P, dims["sumA"] // 16], I16)
            nc.sync.dma_start(
                iaAll[:], idxA_d[None, :, :].to_broadcast(
                    [8, 16, dims["sumA"] // 16]))
            ibAll = const.tile([P, dims["sumB"] // 16], I16)
            nc.sync.dma_start(
                ibAll[:], idxB_d[None, :, :].to_broadcast(
                    [8, 16, dims["sumB"] // 16]))
            dstT = const.tile([P, dims["sumM"]], BF16)
            nc.sync.dma_start(dstT[:], dstT_d[:])
            mskT = const.tile([P, dims["sumM"]], BF16)
            nc.sync.dma_start(mskT[:], mskT_d[:])
            ad2All = const.tile([P, dims["sumM"]], F32)
            nc.sync.dma_start(ad2All[:], ad2T_d[:])
            bvAll = const.tile([P, WINDOWS], BF16)
            nc.sync.dma_start(bvAll[:], bvT_d[:])
            iotaF_t = const.tile([P, P], BF16)
            nc.sync.dma_start(iotaF_t[:], iotaF_d[:].to_broadcast([P, P]))
            giota_t = const.tile([P, N_GRAPHS], BF16)
            nc.sync.dma_start(giota_t[:], giota_d[:].to_broadcast([P, N_GRAPHS]))
            b2_t = const.tile([P, HID], F32)
            nc.sync.dma_start(b2_t[:], b2_d[:].to_broadcast([P, HID]))
            cnt_t = const.tile([N_GRAPHS, 1], F32)
            nc.sync.dma_start(cnt_t[:], cnt_d[:])
            wl_t = const.tile([P, 2], F32)
            nc.vector.memset(wl_t[:], 0.0)
            nc.sync.dma_start(wl_t[:HID, :], Wl_d[:])
            ident_t = const.tile([P, P], F32)
            make_identity(nc, ident_t[:])
            pts = const.tile([P, N_GRAPHS], F32)
            nc.vector.memset(pts[:], 0.0)

            wpool = ctx.enter_context(tc.tile_pool(name="win", bufs=2))
            spool = ctx.enter_context(tc.tile_pool(name="small", bufs=2))
            ps_agg = ctx.enter_context(tc.tile_pool(name="psagg", bufs=2, space="PSUM"))
            ps_pool = ctx.enter_context(tc.tile_pool(name="pspool", bufs=1, space="PSUM"))
            ps_fin = ctx.enter_context(tc.tile_pool(name="psfin", bufs=1, space="PSUM"))

            ppool = ps_pool.tile([N_GRAPHS, HID], F32)

            cA = cB = cM = cE = 0
            for w in range(WINDOWS):
                ma, mb, m = mA[w], mB[w], mW[w]
                Ew = m * P

                v_t = wpool.tile([P, mmax, T2_COLS], BF16, tag="v")
                nc.gpsimd.dma_gather(
                    out_ap=v_t[:, 0:ma, :], in_ap=T2A_d[:],
                    idxs_ap=iaAll[:, cA // 16:(cA + ma * P) // 16],
                    num_idxs=ma * P, num_idxs_reg=ma * P, elem_size=T2_COLS,
                    single_packet=SP_B)
                nc.gpsimd.dma_gather(
                    out_ap=v_t[:, ma:m, :], in_ap=T2B_d[:],
                    idxs_ap=ibAll[:, cB // 16:(cB + mb * P) // 16],
                    num_idxs=mb * P, num_idxs_reg=mb * P, elem_size=T2_COLS,
                    single_packet=SP_B)
                dcol_t = dstT[:, cM:cM + m]
                msk_t = mskT[:, cM:cM + m]
                ad2_t = ad2All[:, cM:cM + m]

                s_t = wpool.tile([P, mmax, P], BF16, tag="s")
                nc.vector.tensor_tensor(
                    s_t[:, :m, :],
                    dcol_t[:, :, None].to_broadcast([P, m, P]),
                    iotaF_t[:, None, :].to_broadcast([P, m, P]),
                    ALU.is_equal)
                zf = spool.tile([P, mmax], F32, tag="zf")
                nc.vector.tensor_tensor(
                    zf[:, :m], v_t[:, :m, 0], ad2_t[:, :], ALU.add)
                zt = spool.tile([P, mmax], F32, tag="zt")
                nc.vector.tensor_scalar_mul(zt[:, :m], zf[:, :m], NEG_SLOPE)
                nc.vector.tensor_tensor(zt[:, :m], zt[:, :m], zf[:, :m], ALU.max)
                ex_t = spool.tile([P, mmax], BF16, tag="ex")
                nc.scalar.activation(ex_t[:, :m], zt[:, :m], AF.Exp)
                nc.vector.tensor_tensor(
                    ex_t[:, :m], ex_t[:, :m], msk_t[:, :], ALU.mult)

                vw_t = wpool.tile([P, mmax, 65], BF16, tag="vw")
                nc.vector.tensor_tensor(
                    vw_t[:, :m, 0:64],
                    v_t[:, :m, 1:65],
                    ex_t[:, :m, None].to_broadcast([P, m, HID]),
                    ALU.mult)
                nc.vector.tensor_copy(vw_t[:, :m, 64:65], ex_t[:, :m, None])

                pagg = ps_agg.tile([P, 65], F32, tag="psagg")
                for j in range(m):
                    nc.tensor.matmul(
                        pagg[:], lhsT=s_t[:, j, :], rhs=vw_t[:, j, :],
                        start=(j == 0), stop=(j == m - 1))
                sden = spool.tile([P, 1], F32, tag="sden")
                nc.vector.tensor_scalar_add(sden[:], pagg[:, 64:65], EPS)
                nc.vector.reciprocal(sden[:], sden[:])
                z2 = spool.tile([P, HID], F32, tag="z2")
                nc.vector.tensor_tensor(
                    z2[:], pagg[:, 0:64], sden[:].to_broadcast([P, HID]),
                    ALU.mult)
                nc.vector.tensor_add(z2[:], z2[:], b2_t[:])
                z2b = spool.tile([P, HID], BF16, tag="z2b")
                nc.scalar.activation(z2b[:], z2[:], AF.Relu)

                pw_t = spool.tile([P, N_GRAPHS], BF16, tag="pw")
                nc.vector.tensor_tensor(
                    pw_t[:], bvAll[:, w, None].to_broadcast([P, N_GRAPHS]),
                    giota_t[:], ALU.is_equal)
                nc.tensor.matmul(
                    ppool[:], lhsT=pw_t[:], rhs=z2b[:],
                    start=(w == 0), stop=(w == WINDOWS - 1))
                cA += ma * P
                cB += mb * P
                cM += m
                cE += Ew

            # pooled partial logits
            crec = spool.tile([N_GRAPHS, 1], F32, tag="crec")
            nc.vector.reciprocal(crec[:], cnt_t[:])
            pooled = spool.tile([N_GRAPHS, HID], F32, tag="pooled")
            nc.vector.tensor_tensor(
                pooled[:], ppool[:], crec[:].to_broadcast([N_GRAPHS, HID]),
                ALU.mult)
            ptp = ps_fin.tile([HID, N_GRAPHS], F32)
            nc.tensor.transpose(ptp[:], pooled[:], ident_t[:N_GRAPHS, :N_GRAPHS])
            nc.vector.tensor_copy(pts[:HID, :], ptp[:])
            plog = ps_fin.tile([N_GRAPHS, 2], F32)
            nc.tensor.matmul(plog[:], lhsT=pts[:], rhs=wl_t[:],
                             start=True, stop=True)
            outs = spool.tile([N_GRAPHS, 2], F32, tag="outs")
            nc.vector.tensor_copy(outs[:], plog[:])
            nc.sync.dma_start(out_d[:], outs[:])

    nc.compile()
    return nc


# ======================================================================
# driver
# ======================================================================

def _run(nc, in_maps, label):
    res = bass_utils.run_bass_kernel_spmd(
        nc, in_maps, core_ids=list(range(NCORES)), trace=TRACE)
    if TRACE:
        LAST_TIMES[label] = res.exec_time_ns
    return res.results


def kernel(x, edge_index, batch, W1, a_src1, a_dst1, b1,
           W2, a_src2, a_dst2, b2, Wl, bl):
    if TRACE:
        try:
            import axon_shim  # noqa: F401
        except ImportError:
            pass

    x = np.asarray(x, np.float32)
    edge_index = np.asarray(edge_index)
    batch = np.asarray(batch)

    key = hashlib.sha1(edge_index.tobytes() + batch.tobytes()).hexdigest()
    if key in _CACHE:
        dims, per_core, nc_d, nc_a, nc_b = _CACHE[key]
    else:
        dims, per_core = _prep(edge_index, batch)
        nc_d = build_dense()
        nc_a = build_phase_a(dims)
        nc_b = build_phase_b(dims)
        _CACHE[key] = (dims, per_core, nc_d, nc_a, nc_b)

    xT, Waug, W2aug = _prep_weights(
        x, np.asarray(W1, np.float32), np.asarray(a_src1, np.float32),
        np.asarray(a_dst1, np.float32), np.asarray(W2, np.float32),
        np.asarray(a_src2, np.float32), np.asarray(a_dst2, np.float32))

    iotaF = np.arange(P, dtype=np.float32).astype(bf16)[None, :]
    iotaC = np.arange(P, dtype=np.float32).astype(bf16)[:, None]
    giota = np.arange(N_GRAPHS, dtype=np.float32).astype(bf16)[None, :]
    b1r = np.asarray(b1, np.float32)[None, :]
    b2r = np.asarray(b2, np.float32)[None, :]
    cnt = np.maximum(
        np.bincount(np.asarray(batch).astype(np.int64), minlength=N_GRAPHS), 1
    ).astype(np.float32)[:, None]
    Wl32 = np.asarray(Wl, np.float32)
    bl32 = np.asarray(bl, np.float32)

    # ---- launch 1: sharded dense1 ----
    NC = NBLK_CORE * P
    start_blk = [(k * OWN) // P for k in range(NCORES)]
    in_maps_d = []
    for k in range(NCORES):
        c0 = start_blk[k] * P
        in_maps_d.append(dict(xTc=np.ascontiguousarray(xT[:, c0:c0 + NC]),
                              Waug=Waug))
    res_d = _run(nc_d, in_maps_d, "dense")

    T1 = np.zeros((NODES_PAD, T1_COLS), bf16)
    for k in range(NCORES):
        r0 = k * OWN
        r1 = min((k + 1) * OWN, NODES_PAD)
        off = r0 - start_blk[k] * P
        T1[r0:r1, :264] = res_d[k]["T1k"][off:off + (r1 - r0), :]
    T1a = np.ascontiguousarray(T1[:TAB_A])
    T1b = np.ascontiguousarray(T1[TAB_A:])

    # ---- launch 2: layer-1 edge phase ----
    in_maps_a = []
    for k in range(NCORES):
        pc = per_core[k]
        ownblk = np.ascontiguousarray(
            T1[k * OWN: k * OWN + OWNPAD, 0:8])
        in_maps_a.append(dict(
            T1a=T1a, T1b=T1b, W2aug=W2aug,
            idxA=pc["idxA"], idxB=pc["idxB"],
            dstcol=pc["dstcol"], dstT=pc["dstT"], mskT=pc["mskT"],
            ownblk=ownblk,
            iotaF=iotaF, iotaC=iotaC, b1r=b1r,
        ))
    res_a = _run(nc_a, in_maps_a, "phase_a")

    T2 = np.zeros((NODES_PAD, T2_COLS), bf16)
    for k in range(NCORES):
        T2[k * OWN:(k + 1) * OWN, 0:65] = \
            res_a[k]["T2own"][:OWN, :].astype(bf16)
    T2A = np.ascontiguousarray(T2[:TAB_A])
    T2B = np.ascontiguousarray(T2[TAB_A:])

    # ---- launch 3: layer-2 edge phase + pool ----
    in_maps_b = []
    for k in range(NCORES):
        pc = per_core[k]
        ad2T = np.ascontiguousarray(
            res_a[k]["ad2"].reshape(-1, P).T)
        in_maps_b.append(dict(
            T2A=T2A, T2B=T2B,
            idxA=pc["idxA"], idxB=pc["idxB"],
            dstT=pc["dstT"], mskT=pc["mskT"],
            ad2T=ad2T,
            iotaF=iotaF, giota=giota,
            bvT=pc["bvT"], b2r=b2r, cnt=cnt, Wl=Wl32,
        ))
    res_b = _run(nc_b, in_maps_b, "phase_b")

    out = np.zeros((N_GRAPHS, 2), np.float32)
    for k in range(NCORES):
        out += res_b[k]["partial"]
    out += bl32[None, :]
    return out



# revision 5
# speedup vs baseline: 1.2381x; 1.2381x over previous
"""GAT (2-layer, 4-head + 1-head) + global mean pool + linear head on 8 TRN2 cores.

v2 design (vs baseline): dst windows (392 blocks of 128 nodes) are
load-balanced across cores (LPT) and slot-sorted so the SPMD per-slot
padding is small. Each edge phase uses TWO dma_gathers per window on
4 parallel SWDGE queues (num_swdge_queues=4 -> Q7 core pairs work
concurrently, ~2.5ns/idx vs 8.3ns on one queue):
  g1: per-edge source row   [as|ad|h1|junk] (768B) by src index
  g2: per-edge dst attn row [as|ad|...]     (256B, elem_step=384) by dst
so the per-edge attention terms (ad for layer 1, ad2 for layer 2) come
from DMA instead of per-edge one-hot matmuls on the Tensor engine.
Single node table with int16 indices via midpoint bias (idx = node - 25088).

Launch 1 (dense): T1[as|ad|h1] = x @ Waug per 1/8 node slice; host
  stitches the [50176, 384] bf16 table (cols 264:384 zero).
Launch 2 (layer-1 edge phase): per-window softmax aggregation via
  indicator matmuls + dense2 -> T2own rows [as2|h2|ad2].
Launch 3 (layer-2 edge phase): same pattern on the [50176, 128] T2
  table -> global mean pool partials -> partial [64,2] logits; host
  sums partials + bl.
"""

import contextlib
import hashlib
import os
import numpy as np
import ml_dtypes

import concourse.bass as bass
import concourse.mybir as mybir
import concourse.tile as tile
from concourse import bacc
from concourse import bass_utils
from concourse.masks import make_identity

bf16 = ml_dtypes.bfloat16
F32 = mybir.dt.float32
BF16 = mybir.dt.bfloat16
I16 = mybir.dt.int16
AF = mybir.ActivationFunctionType
ALU = mybir.AluOpType

# ---- problem constants ----
N_NODES = 50000
N_GRAPHS = 64
F_IN = 500
F_IN_PAD = 512
H1 = 256          # heads*hid layer 1
HEADS = 4
HID = 64
NEG_SLOPE = 0.2
NCORES = 8
P = 128
NODES_PAD = 50176                # 392*128
NWIN = NODES_PAD // P            # 392 global windows
WINDOWS = NWIN // NCORES         # 49 slots per core
MID = NODES_PAD // 2             # 25088: int16 idx bias
T1_COLS = 384                    # bf16 row: [as(4) | ad(4) | h1(256) | 0(120)]
T2_COLS = 128                    # bf16 row: [as2(1) | h2(64) | ad2(1) | 0(62)]
OWNPAD = WINDOWS * P             # 6272 own nodes per core
NBLK_CORE = WINDOWS              # dense: 49 node blocks per core
EPS = 1e-16
NQ = 4                           # SWDGE queues

TRACE = bool(int(os.environ.get("KERNEL_TRACE", "0")))
LAST_TIMES = {}

_CACHE = {}


# ======================================================================
# host preprocessing
# ======================================================================

def _wrap_idx(idx, L):
    pad = np.zeros(L, np.int16)
    pad[: len(idx)] = idx
    return pad.reshape(L // 16, 16).T.astype(np.int16)  # [16, L/16]


def _prep(edge_index, batch):
    src = np.concatenate([edge_index[0], np.arange(N_NODES, dtype=np.int64)])
    dst = np.concatenate([edge_index[1], np.arange(N_NODES, dtype=np.int64)])
    src = src.astype(np.int32)
    dst = dst.astype(np.int32)

    win = dst >> 7
    counts = np.bincount(win, minlength=NWIN)

    # LPT assignment of the 392 windows to 8 cores, 49 each
    order = np.argsort(-counts, kind="stable")
    loads = np.zeros(NCORES, np.int64)
    nwin = np.zeros(NCORES, np.int64)
    assign = np.zeros(NWIN, np.int64)
    for w in order:
        k = min((kk for kk in range(NCORES) if nwin[kk] < WINDOWS),
                key=lambda kk: loads[kk])
        assign[w] = k
        loads[k] += counts[w]
        nwin[k] += 1
    # per-core windows sorted by count desc -> slot j balanced across cores
    core_wins = []
    for k in range(NCORES):
        ws = np.where(assign == k)[0]
        ws = ws[np.argsort(-counts[ws], kind="stable")]
        core_wins.append(ws)
    core_wins = np.stack(core_wins)          # [NCORES, WINDOWS] global win id

    mW = [max(1, int(np.ceil(max(counts[core_wins[k][j]]
                                 for k in range(NCORES)) / P)))
          for j in range(WINDOWS)]
    dims = dict(mW=mW, sumM=sum(mW), sumE=sum(mW) * P, mmax=max(mW))

    # bucket edges by window
    eorder = np.argsort(win, kind="stable")
    estart = np.searchsorted(win[eorder], np.arange(NWIN + 1))

    batch_pad = np.full(NODES_PAD, -1.0, np.float32)
    batch_pad[:N_NODES] = batch.astype(np.float32)

    per_core = []
    for k in range(NCORES):
        sidx = np.zeros((16, dims["sumE"] // 16), np.int16)
        didx = np.zeros((16, dims["sumE"] // 16), np.int16)
        dstw = np.full((P, dims["sumM"]), -1.0, bf16)
        mskw = np.zeros((P, dims["sumM"]), bf16)
        bv = np.zeros((WINDOWS, P), np.float32)
        cE = cM = 0
        for j in range(WINDOWS):
            w = core_wins[k][j]
            sl = eorder[estart[w]:estart[w + 1]]
            s, d = src[sl], dst[sl]
            n = len(s)
            L = mW[j] * P
            sidx[:, cE // 16:(cE + L) // 16] = _wrap_idx(
                (s - MID).astype(np.int16), L)
            didx[:, cE // 16:(cE + L) // 16] = _wrap_idx(
                (d - MID).astype(np.int16), L)
            dv = np.full(L, -1.0, np.float32)
            dv[:n] = d - (w << 7)
            mv = np.zeros(L, np.float32)
            mv[:n] = 1.0
            dstw[:, cM:cM + mW[j]] = dv.reshape(mW[j], P).T.astype(bf16)
            mskw[:, cM:cM + mW[j]] = mv.reshape(mW[j], P).T.astype(bf16)
            bv[j] = batch_pad[w << 7:(w + 1) << 7]
            cE += L
            cM += mW[j]
        per_core.append(dict(
            sidx=sidx, didx=didx, dstw=dstw, mskw=mskw,
            bvT=np.ascontiguousarray(bv.T.astype(bf16))))
    return dims, core_wins, per_core


def _prep_weights(x, W1, a_src1, a_dst1, W2, a_src2, a_dst2):
    xT = np.zeros((F_IN_PAD, NODES_PAD), bf16)
    xT[:F_IN, :N_NODES] = x.T.astype(bf16)

    Asrc = np.zeros((H1, HEADS), np.float32)
    Adst = np.zeros((H1, HEADS), np.float32)
    for h in range(HEADS):
        Asrc[h * HID:(h + 1) * HID, h] = a_src1[h]
        Adst[h * HID:(h + 1) * HID, h] = a_dst1[h]
    Waug = np.zeros((F_IN_PAD, 8 + H1), np.float32)
    Waug[:F_IN, 0:4] = W1 @ Asrc
    Waug[:F_IN, 4:8] = W1 @ Adst
    Waug[:F_IN, 8:] = W1
    Waug = Waug.astype(bf16)

    W2aug = np.zeros((H1, 66), np.float32)   # [as2 | h2(64) | ad2]
    W2aug[:, 0] = W2 @ a_src2[0]
    W2aug[:, 1:65] = W2
    W2aug[:, 65] = W2 @ a_dst2[0]
    W2aug = W2aug.astype(bf16)
    return xT, Waug, W2aug


# ======================================================================
# launch 1: dense1 (sharded across cores)
# ======================================================================

def build_dense():
    nc = bacc.Bacc("TRN2", target_bir_lowering=False, debug=False,
                   num_swdge_queues=NQ)

    NC = NBLK_CORE * P  # 6272 nodes per core
    xTc_d = nc.dram_tensor("xTc", [F_IN_PAD, NC], BF16, kind="ExternalInput")
    Waug_d = nc.dram_tensor("Waug", [F_IN_PAD, 264], BF16, kind="ExternalInput")
    T1k_d = nc.dram_tensor("T1k", [NC, 264], BF16, kind="ExternalOutput")

    with tile.TileContext(nc) as tc:
        ctx = contextlib.ExitStack()
        with ctx:
            const = ctx.enter_context(tc.tile_pool(name="const", bufs=1))
            waug_t = const.tile([P, 4, 264], BF16)
            nc.sync.dma_start(waug_t[:], Waug_d[:].rearrange("(ko p) c -> p ko c", p=P))

            CH = 10  # node blocks per xT chunk
            with tc.tile_pool(name="dense", bufs=3) as dpool, \
                 tc.tile_pool(name="dpsum", bufs=4, space="PSUM") as dps:
                for c0 in range(0, NBLK_CORE, CH):
                    nchunk = min(CH, NBLK_CORE - c0) * P
                    xt_t = dpool.tile([P, 4, CH * P], BF16, tag="xt")
                    nc.sync.dma_start(
                        xt_t[:, :, :nchunk],
                        xTc_d[:].rearrange("(ko p) n -> p ko n", p=P)[
                            :, :, c0 * P: c0 * P + nchunk],
                    )
                    for b in range(nchunk // P):
                        ps = dps.tile([P, 264], F32, tag="dps")
                        for ko in range(4):
                            nc.tensor.matmul(
                                ps[:],
                                lhsT=xt_t[:, ko, b * P:(b + 1) * P],
                                rhs=waug_t[:, ko, :],
                                start=(ko == 0),
                                stop=(ko == 3),
                            )
                        t1_t = dpool.tile([P, 264], BF16, tag="t1")
                        nc.scalar.copy(t1_t[:], ps[:])
                        nb = c0 + b
                        nc.sync.dma_start(
                            T1k_d[nb * P:(nb + 1) * P, :], t1_t[:])

    nc.compile()
    return nc


# ======================================================================
# launch 2: layer-1 edge phase + dense2
# ======================================================================

def build_phase_a(dims):
    mW = dims["mW"]
    mmax = dims["mmax"]
    nc = bacc.Bacc("TRN2", target_bir_lowering=False, debug=False,
                   num_swdge_queues=NQ)

    T1_d = nc.dram_tensor("T1", [NODES_PAD, T1_COLS], BF16, kind="ExternalInput")
    T1a_d = nc.dram_tensor("T1a", [NODES_PAD, P], BF16, kind="ExternalInput")
    sidx_d = nc.dram_tensor("sidx", [16, dims["sumE"] // 16], I16, kind="ExternalInput")
    didx_d = nc.dram_tensor("didx", [16, dims["sumE"] // 16], I16, kind="ExternalInput")
    dstw_d = nc.dram_tensor("dstw", [P, dims["sumM"]], BF16, kind="ExternalInput")
    mskw_d = nc.dram_tensor("mskw", [P, dims["sumM"]], BF16, kind="ExternalInput")
    iotaF_d = nc.dram_tensor("iotaF", [1, P], BF16, kind="ExternalInput")
    b1_d = nc.dram_tensor("b1r", [1, H1], F32, kind="ExternalInput")
    W2aug_d = nc.dram_tensor("W2aug", [H1, 66], BF16, kind="ExternalInput")

    T2own_d = nc.dram_tensor("T2own", [OWNPAD, 66], F32, kind="ExternalOutput")

    T1mid = T1_d[MID:, :]
    T1attn = T1a_d[MID:, :]

    with tile.TileContext(nc) as tc:
        ctx = contextlib.ExitStack()
        with ctx:
            const = ctx.enter_context(tc.tile_pool(name="const", bufs=1))
            # index loads FIRST (gathers wait on them)
            iaAll = const.tile([P, dims["sumE"] // 16], I16)
            nc.sync.dma_start(
                iaAll[:], sidx_d[None, :, :].to_broadcast(
                    [8, 16, dims["sumE"] // 16]))
            ibAll = const.tile([P, dims["sumE"] // 16], I16)
            nc.sync.dma_start(
                ibAll[:], didx_d[None, :, :].to_broadcast(
                    [8, 16, dims["sumE"] // 16]))
            dstw = const.tile([P, dims["sumM"]], BF16)
            nc.sync.dma_start(dstw[:], dstw_d[:])
            mskw = const.tile([P, dims["sumM"]], BF16)
            nc.sync.dma_start(mskw[:], mskw_d[:])
            w2aug_t = const.tile([P, 2, 66], BF16)
            nc.sync.dma_start(w2aug_t[:], W2aug_d[:].rearrange("(ko p) c -> p ko c", p=P))
            iotaF_t = const.tile([P, P], BF16)
            nc.sync.dma_start(iotaF_t[:], iotaF_d[:].to_broadcast([P, P]))
            b1_t = const.tile([P, H1], F32)
            nc.sync.dma_start(b1_t[:], b1_d[:].to_broadcast([P, H1]))
            ident_t = const.tile([P, P], F32)
            make_identity(nc, ident_t[:])

            wpool = ctx.enter_context(tc.tile_pool(name="win", bufs=3))
            spool = ctx.enter_context(tc.tile_pool(name="small", bufs=2))
            ps_agg = ctx.enter_context(tc.tile_pool(name="psagg", bufs=2, space="PSUM"))
            ps_z1t = ctx.enter_context(tc.tile_pool(name="psz1t", bufs=2, space="PSUM"))
            ps_h2 = ctx.enter_context(tc.tile_pool(name="psh2", bufs=2, space="PSUM"))

            cE = cM = 0
            for w in range(WINDOWS):
                m = mW[w]
                rows = P

                v_t = wpool.tile([P, mmax, T1_COLS], BF16, tag="v")
                nc.gpsimd.dma_gather(
                    out_ap=v_t[:, 0:m, :], in_ap=T1mid,
                    idxs_ap=iaAll[:, cE // 16:(cE + m * P) // 16],
                    num_idxs=m * P, num_idxs_reg=m * P, elem_size=T1_COLS,
                    single_packet=False, queue_num=(2 * w) % NQ)
                a_t = wpool.tile([P, mmax, P], BF16, tag="a")
                nc.gpsimd.dma_gather(
                    out_ap=a_t[:, 0:m, :], in_ap=T1attn,
                    idxs_ap=ibAll[:, cE // 16:(cE + m * P) // 16],
                    num_idxs=m * P, num_idxs_reg=m * P, elem_size=P,
                    single_packet=False, queue_num=(2 * w + 1) % NQ)

                dcol_t = dstw[:, cM:cM + m]
                msk_t = mskw[:, cM:cM + m]

                # --- S (edge-major indicator) ---
                s_t = wpool.tile([P, mmax, P], BF16, tag="s")
                nc.vector.tensor_tensor(
                    s_t[:, :m, :],
                    dcol_t[:, :, None].to_broadcast([P, m, P]),
                    iotaF_t[:, None, :].to_broadcast([P, m, P]),
                    ALU.is_equal)

                # --- ex = exp(lrelu(as + ad)) * mask ---
                zf = spool.tile([P, mmax, 4], F32, tag="zf")
                nc.vector.tensor_tensor(
                    zf[:, :m, :], v_t[:, :m, 0:4], a_t[:, :m, 4:8], ALU.add)
                zt = spool.tile([P, mmax, 4], F32, tag="zt")
                nc.vector.tensor_scalar_mul(zt[:, :m, :], zf[:, :m, :], NEG_SLOPE)
                nc.vector.tensor_tensor(zt[:, :m, :], zt[:, :m, :], zf[:, :m, :],
                                        ALU.max)
                ex_t = spool.tile([P, mmax, 4], BF16, tag="ex")
                nc.scalar.activation(ex_t[:, :m, :], zt[:, :m, :], AF.Exp)
                nc.vector.tensor_tensor(
                    ex_t[:, :m, :], ex_t[:, :m, :],
                    msk_t[:, :, None].to_broadcast([P, m, 4]), ALU.mult)

                # --- Vw = [h*ex | ex] ---
                vw_t = wpool.tile([P, mmax, 260], BF16, tag="vw")
                nc.vector.tensor_tensor(
                    vw_t[:, :m, 0:256].rearrange("p m (h c) -> p m h c", h=HEADS),
                    v_t[:, :m, 8:264].rearrange("p m (h c) -> p m h c", h=HEADS),
                    ex_t[:, :m, :, None].to_broadcast([P, m, HEADS, HID]),
                    ALU.mult)
                nc.vector.tensor_copy(vw_t[:, :m, 256:260], ex_t[:, :m, :])

                # --- aggregate ---
                pagg = ps_agg.tile([P, 260], F32, tag="psagg")
                for j in range(m):
                    nc.tensor.matmul(
                        pagg[:], lhsT=s_t[:, j, :], rhs=vw_t[:, j, :],
                        start=(j == 0), stop=(j == m - 1))
                # --- out1 = agg / s + b1 ; z1 = relu ---
                sden = spool.tile([P, 4], F32, tag="sden")
                nc.vector.tensor_scalar_add(sden[:], pagg[:, 256:260], EPS)
                nc.vector.reciprocal(sden[:], sden[:])
                z1 = spool.tile([P, H1], F32, tag="z1")
                nc.vector.tensor_tensor(
                    z1[:].rearrange("p (h c) -> p h c", h=HEADS),
                    pagg[:, 0:256].rearrange("p (h c) -> p h c", h=HEADS),
                    sden[:, :, None].to_broadcast([P, HEADS, HID]),
                    ALU.mult)
                nc.vector.tensor_add(z1[:], z1[:], b1_t[:])
                nc.scalar.activation(z1[:], z1[:], AF.Relu)

                # --- dense 2: [as2|h2|ad2] = z1 @ W2aug ---
                z1t = spool.tile([P, 2, P], BF16, tag="z1t")
                for hh in range(2):
                    pzt = ps_z1t.tile([P, P], F32, tag="psz1t")
                    nc.tensor.transpose(
                        pzt[:], z1[:, hh * P:(hh + 1) * P], ident_t[:])
                    nc.scalar.copy(z1t[:, hh, :], pzt[:])
                ph2 = ps_h2.tile([P, 66], F32, tag="psh2")
                for hh in range(2):
                    nc.tensor.matmul(
                        ph2[:], lhsT=z1t[:, hh, :], rhs=w2aug_t[:, hh, :],
                        start=(hh == 0), stop=(hh == 1))
                t2_t = spool.tile([P, 66], F32, tag="t2")
                nc.vector.tensor_copy(t2_t[:], ph2[:])
                nc.sync.dma_start(
                    T2own_d[w * P: w * P + rows, :], t2_t[:rows, :])

                cE += m * P
                cM += m

    nc.compile()
    return nc


# ======================================================================
# launch 3: layer-2 edge phase + pool + logits
# ======================================================================

def build_phase_b(dims):
    mW = dims["mW"]
    mmax = dims["mmax"]
    nc = bacc.Bacc("TRN2", target_bir_lowering=False, debug=False,
                   num_swdge_queues=NQ)

    T2_d = nc.dram_tensor("T2", [NODES_PAD, T2_COLS], BF16, kind="ExternalInput")
    sidx_d = nc.dram_tensor("sidx", [16, dims["sumE"] // 16], I16, kind="ExternalInput")
    didx_d = nc.dram_tensor("didx", [16, dims["sumE"] // 16], I16, kind="ExternalInput")
    dstw_d = nc.dram_tensor("dstw", [P, dims["sumM"]], BF16, kind="ExternalInput")
    mskw_d = nc.dram_tensor("mskw", [P, dims["sumM"]], BF16, kind="ExternalInput")
    iotaF_d = nc.dram_tensor("iotaF", [1, P], BF16, kind="ExternalInput")
    giota_d = nc.dram_tensor("giota", [1, N_GRAPHS], BF16, kind="ExternalInput")
    bvT_d = nc.dram_tensor("bvT", [P, WINDOWS], BF16, kind="ExternalInput")
    b2_d = nc.dram_tensor("b2r", [1, HID], F32, kind="ExternalInput")
    cnt_d = nc.dram_tensor("cnt", [N_GRAPHS, 1], F32, kind="ExternalInput")
    Wl_d = nc.dram_tensor("Wl", [HID, 2], F32, kind="ExternalInput")

    out_d = nc.dram_tensor("partial", [N_GRAPHS, 2], F32, kind="ExternalOutput")

    T2mid = T2_d[MID:, :]

    with tile.TileContext(nc) as tc:
        ctx = contextlib.ExitStack()
        with ctx:
            const = ctx.enter_context(tc.tile_pool(name="const", bufs=1))
            iaAll = const.tile([P, dims["sumE"] // 16], I16)
            nc.sync.dma_start(
                iaAll[:], sidx_d[None, :, :].to_broadcast(
                    [8, 16, dims["sumE"] // 16]))
            ibAll = const.tile([P, dims["sumE"] // 16], I16)
            nc.sync.dma_start(
                ibAll[:], didx_d[None, :, :].to_broadcast(
                    [8, 16, dims["sumE"] // 16]))
            dstw = const.tile([P, dims["sumM"]], BF16)
            nc.sync.dma_start(dstw[:], dstw_d[:])
            mskw = const.tile([P, dims["sumM"]], BF16)
            nc.sync.dma_start(mskw[:], mskw_d[:])
            bvAll = const.tile([P, WINDOWS], BF16)
            nc.sync.dma_start(bvAll[:], bvT_d[:])
            iotaF_t = const.tile([P, P], BF16)
            nc.sync.dma_start(iotaF_t[:], iotaF_d[:].to_broadcast([P, P]))
            giota_t = const.tile([P, N_GRAPHS], BF16)
            nc.sync.dma_start(giota_t[:], giota_d[:].to_broadcast([P, N_GRAPHS]))
            b2_t = const.tile([P, HID], F32)
            nc.sync.dma_start(b2_t[:], b2_d[:].to_broadcast([P, HID]))
            cnt_t = const.tile([N_GRAPHS, 1], F32)
            nc.sync.dma_start(cnt_t[:], cnt_d[:])
            wl_t = const.tile([P, 2], F32)
            nc.vector.memset(wl_t[:], 0.0)
            nc.sync.dma_start(wl_t[:HID, :], Wl_d[:])
            ident_t = const.tile([P, P], F32)
            make_identity(nc, ident_t[:])
            pts = const.tile([P, N_GRAPHS], F32)
            nc.vector.memset(pts[:], 0.0)

            wpool = ctx.enter_context(tc.tile_pool(name="win", bufs=3))
            spool = ctx.enter_context(tc.tile_pool(name="small", bufs=2))
            ps_agg = ctx.enter_context(tc.tile_pool(name="psagg", bufs=2, space="PSUM"))
            ps_pool = ctx.enter_context(tc.tile_pool(name="pspool", bufs=1, space="PSUM"))
            ps_fin = ctx.enter_context(tc.tile_pool(name="psfin", bufs=1, space="PSUM"))

            ppool = ps_pool.tile([N_GRAPHS, HID], F32)

            cE = cM = 0
            for w in range(WINDOWS):
                m = mW[w]

                v_t = wpool.tile([P, mmax, T2_COLS], BF16, tag="v")
                nc.gpsimd.dma_gather(
                    out_ap=v_t[:, 0:m, :], in_ap=T2mid,
                    idxs_ap=iaAll[:, cE // 16:(cE + m * P) // 16],
                    num_idxs=m * P, num_idxs_reg=m * P, elem_size=T2_COLS,
                    single_packet=False, queue_num=(2 * w) % NQ)
                a_t = wpool.tile([P, mmax, T2_COLS], BF16, tag="a")
                nc.gpsimd.dma_gather(
                    out_ap=a_t[:, 0:m, :], in_ap=T2mid,
                    idxs_ap=ibAll[:, cE // 16:(cE + m * P) // 16],
                    num_idxs=m * P, num_idxs_reg=m * P, elem_size=T2_COLS,
                    single_packet=False, queue_num=(2 * w + 1) % NQ)

                dcol_t = dstw[:, cM:cM + m]
                msk_t = mskw[:, cM:cM + m]

                s_t = wpool.tile([P, mmax, P], BF16, tag="s")
                nc.vector.tensor_tensor(
                    s_t[:, :m, :],
                    dcol_t[:, :, None].to_broadcast([P, m, P]),
                    iotaF_t[:, None, :].to_broadcast([P, m, P]),
                    ALU.is_equal)
                zf = spool.tile([P, mmax], F32, tag="zf")
                nc.vector.tensor_tensor(
                    zf[:, :m], v_t[:, :m, 0], a_t[:, :m, 65], ALU.add)
                zt = spool.tile([P, mmax], F32, tag="zt")
                nc.vector.tensor_scalar_mul(zt[:, :m], zf[:, :m], NEG_SLOPE)
                nc.vector.tensor_tensor(zt[:, :m], zt[:, :m], zf[:, :m], ALU.max)
                ex_t = spool.tile([P, mmax], BF16, tag="ex")
                nc.scalar.activation(ex_t[:, :m], zt[:, :m], AF.Exp)
                nc.vector.tensor_tensor(
                    ex_t[:, :m], ex_t[:, :m], msk_t[:, :], ALU.mult)

                vw_t = wpool.tile([P, mmax, 65], BF16, tag="vw")
                nc.vector.tensor_tensor(
                    vw_t[:, :m, 0:64],
                    v_t[:, :m, 1:65],
                    ex_t[:, :m, None].to_broadcast([P, m, HID]),
                    ALU.mult)
                nc.vector.tensor_copy(vw_t[:, :m, 64:65], ex_t[:, :m, None])

                pagg = ps_agg.tile([P, 65], F32, tag="psagg")
                for j in range(m):
                    nc.tensor.matmul(
                        pagg[:], lhsT=s_t[:, j, :], rhs=vw_t[:, j, :],
                        start=(j == 0), stop=(j == m - 1))
                sden = spool.tile([P, 1], F32, tag="sden")
                nc.vector.tensor_scalar_add(sden[:], pagg[:, 64:65], EPS)
                nc.vector.reciprocal(sden[:], sden[:])
                z2 = spool.tile([P, HID], F32, tag="z2")
                nc.vector.tensor_tensor(
                    z2[:], pagg[:, 0:64], sden[:].to_broadcast([P, HID]),
                    ALU.mult)
                nc.vector.tensor_add(z2[:], z2[:], b2_t[:])
                z2b = spool.tile([P, HID], BF16, tag="z2b")
                nc.scalar.activation(z2b[:], z2[:], AF.Relu)

                pw_t = spool.tile([P, N_GRAPHS], BF16, tag="pw")
                nc.vector.tensor_tensor(
                    pw_t[:], bvAll[:, w, None].to_broadcast([P, N_GRAPHS]),
                    giota_t[:], ALU.is_equal)
                nc.tensor.matmul(
                    ppool[:], lhsT=pw_t[:], rhs=z2b[:],
                    start=(w == 0), stop=(w == WINDOWS - 1))
                cE += m * P
                cM += m

            # pooled partial logits
            crec = spool.tile([N_GRAPHS, 1], F32, tag="crec")
            nc.vector.reciprocal(crec[:], cnt_t[:])
            pooled = spool.tile([N_GRAPHS, HID], F32, tag="pooled")
            nc.vector.tensor_tensor(
                pooled[:], ppool[:], crec[:].to_broadcast([N_GRAPHS, HID]),
                ALU.mult)
            ptp = ps_fin.tile([HID, N_GRAPHS], F32)
            nc.tensor.transpose(ptp[:], pooled[:], ident_t[:N_GRAPHS, :N_GRAPHS])
            nc.vector.tensor_copy(pts[:HID, :], ptp[:])
            plog = ps_fin.tile([N_GRAPHS, 2], F32)
            nc.tensor.matmul(plog[:], lhsT=pts[:], rhs=wl_t[:],
                             start=True, stop=True)
            outs = spool.tile([N_GRAPHS, 2], F32, tag="outs")
            nc.vector.tensor_copy(outs[:], plog[:])
            nc.sync.dma_start(out_d[:], outs[:])

    nc.compile()
    return nc


# ======================================================================
# driver
# ======================================================================

def _run(nc, in_maps, label):
    res = bass_utils.run_bass_kernel_spmd(
        nc, in_maps, core_ids=list(range(NCORES)), trace=TRACE)
    if TRACE:
        LAST_TIMES[label] = res.exec_time_ns
    return res.results


def kernel(x, edge_index, batch, W1, a_src1, a_dst1, b1,
           W2, a_src2, a_dst2, b2, Wl, bl):
    if TRACE:
        try:
            import axon_shim  # noqa: F401
        except ImportError:
            pass

    x = np.asarray(x, np.float32)
    edge_index = np.asarray(edge_index)
    batch = np.asarray(batch)

    key = hashlib.sha1(edge_index.tobytes() + batch.tobytes()).hexdigest()
    if key in _CACHE:
        dims, core_wins, per_core, nc_d, nc_a, nc_b = _CACHE[key]
    else:
        dims, core_wins, per_core = _prep(edge_index, batch)
        nc_d = build_dense()
        nc_a = build_phase_a(dims)
        nc_b = build_phase_b(dims)
        _CACHE[key] = (dims, core_wins, per_core, nc_d, nc_a, nc_b)

    xT, Waug, W2aug = _prep_weights(
        x, np.asarray(W1, np.float32), np.asarray(a_src1, np.float32),
        np.asarray(a_dst1, np.float32), np.asarray(W2, np.float32),
        np.asarray(a_src2, np.float32), np.asarray(a_dst2, np.float32))

    iotaF = np.arange(P, dtype=np.float32).astype(bf16)[None, :]
    giota = np.arange(N_GRAPHS, dtype=np.float32).astype(bf16)[None, :]
    b1r = np.asarray(b1, np.float32)[None, :]
    b2r = np.asarray(b2, np.float32)[None, :]
    cnt = np.maximum(
        np.bincount(np.asarray(batch).astype(np.int64), minlength=N_GRAPHS), 1
    ).astype(np.float32)[:, None]
    Wl32 = np.asarray(Wl, np.float32)
    bl32 = np.asarray(bl, np.float32)

    # ---- launch 1: sharded dense1 ----
    NC = NBLK_CORE * P
    in_maps_d = []
    for k in range(NCORES):
        c0 = k * NC
        in_maps_d.append(dict(xTc=np.ascontiguousarray(xT[:, c0:c0 + NC]),
                              Waug=Waug))
    res_d = _run(nc_d, in_maps_d, "dense")

    T1 = np.zeros((NODES_PAD, T1_COLS), bf16)
    for k in range(NCORES):
        T1[k * NC:(k + 1) * NC, 0:264] = res_d[k]["T1k"]
    T1a = np.zeros((NODES_PAD, P), bf16)
    T1a[:, 0:8] = T1[:, 0:8]

    # ---- launch 2: layer-1 edge phase ----
    in_maps_a = []
    for k in range(NCORES):
        pc = per_core[k]
        in_maps_a.append(dict(
            T1=T1, T1a=T1a, sidx=pc["sidx"], didx=pc["didx"],
            dstw=pc["dstw"], mskw=pc["mskw"],
            iotaF=iotaF, b1r=b1r, W2aug=W2aug,
        ))
    res_a = _run(nc_a, in_maps_a, "phase_a")

    T2 = np.zeros((NODES_PAD, T2_COLS), bf16)
    for k in range(NCORES):
        t2k = res_a[k]["T2own"].astype(bf16)          # [OWNPAD, 66]
        wins = core_wins[k]
        T2[(wins[:, None] << 7) + np.arange(P)[None, :], 0:66] = \
            t2k.reshape(WINDOWS, P, 66)

    # ---- launch 3: layer-2 edge phase + pool ----
    in_maps_b = []
    for k in range(NCORES):
        pc = per_core[k]
        in_maps_b.append(dict(
            T2=T2, sidx=pc["sidx"], didx=pc["didx"],
            dstw=pc["dstw"], mskw=pc["mskw"],
            iotaF=iotaF, giota=giota,
            bvT=pc["bvT"], b2r=b2r, cnt=cnt, Wl=Wl32,
        ))
    res_b = _run(nc_b, in_maps_b, "phase_b")

    out = np.zeros((N_GRAPHS, 2), np.float32)
    for k in range(NCORES):
        out += res_b[k]["partial"]
    out += bl32[None, :]
    return out


# revision 12
# speedup vs baseline: 1.4923x; 1.2053x over previous
"""GAT (2-layer, 4-head + 1-head) + global mean pool + linear head on 8 TRN2 cores.

v2 design (vs baseline): dst windows (392 blocks of 128 nodes) are
load-balanced across cores (LPT) and slot-sorted so the SPMD per-slot
padding is small. Each edge phase uses TWO dma_gathers per window on
4 parallel SWDGE queues (num_swdge_queues=4 -> Q7 core pairs work
concurrently, ~2.5ns/idx vs 8.3ns on one queue):
  g1: per-edge source row   [as|ad|h1|junk] (768B) by src index
  g2: per-edge dst attn row [as|ad|...]     (256B, elem_step=384) by dst
so the per-edge attention terms (ad for layer 1, ad2 for layer 2) come
from DMA instead of per-edge one-hot matmuls on the Tensor engine.
Single node table with int16 indices via midpoint bias (idx = node - 25088).

Launch 1 (dense): T1[as|ad|h1] = x @ Waug per 1/8 node slice; host
  stitches the [50176, 384] bf16 table (cols 264:384 zero).
Launch 2 (layer-1 edge phase): per-window softmax aggregation via
  indicator matmuls + dense2 -> T2own rows [as2|h2|ad2].
Launch 3 (layer-2 edge phase): same pattern on the [50176, 128] T2
  table -> global mean pool partials -> partial [64,2] logits; host
  sums partials + bl.
"""

import contextlib
import hashlib
import os
import numpy as np
import ml_dtypes

import concourse.bass as bass
import concourse.mybir as mybir
import concourse.tile as tile
from concourse import bacc
from concourse import bass_utils
from concourse.masks import make_identity

bf16 = ml_dtypes.bfloat16
F32 = mybir.dt.float32
BF16 = mybir.dt.bfloat16
I16 = mybir.dt.int16
AF = mybir.ActivationFunctionType
ALU = mybir.AluOpType

# ---- problem constants ----
N_NODES = 50000
N_GRAPHS = 64
F_IN = 500
F_IN_PAD = 512
H1 = 256          # heads*hid layer 1
HEADS = 4
HID = 64
NEG_SLOPE = 0.2
NCORES = 8
P = 128
NODES_PAD = 50176                # 392*128
NWIN = NODES_PAD // P            # 392 global windows
WINDOWS = NWIN // NCORES         # 49 slots per core
MID = NODES_PAD // 2             # 25088: int16 idx bias
T1_COLS = 384                    # bf16 row: [as(4) | ad(4) | h1(256) | 0(120)]
T2_COLS = 128                    # bf16 row: [as2(1) | h2(64) | ad2(1) | 0(62)]
OWNPAD = WINDOWS * P             # 6272 own nodes per core
NBLK_CORE = WINDOWS              # dense: 49 node blocks per core
EPS = 1e-16
NQ = 4                           # SWDGE queues

TRACE = bool(int(os.environ.get("KERNEL_TRACE", "0")))
LAST_TIMES = {}

_CACHE = {}


# ======================================================================
# host preprocessing
# ======================================================================

def _wrap_idx(idx, L):
    pad = np.zeros(L, np.int16)
    pad[: len(idx)] = idx
    return pad.reshape(L // 16, 16).T.astype(np.int16)  # [16, L/16]


def _prep(edge_index, batch):
    src = np.concatenate([edge_index[0], np.arange(N_NODES, dtype=np.int64)])
    dst = np.concatenate([edge_index[1], np.arange(N_NODES, dtype=np.int64)])
    src = src.astype(np.int32)
    dst = dst.astype(np.int32)

    win = dst >> 7
    counts = np.bincount(win, minlength=NWIN)

    # LPT assignment of the 392 windows to 8 cores, 49 each
    order = np.argsort(-counts, kind="stable")
    loads = np.zeros(NCORES, np.int64)
    nwin = np.zeros(NCORES, np.int64)
    assign = np.zeros(NWIN, np.int64)
    for w in order:
        k = min((kk for kk in range(NCORES) if nwin[kk] < WINDOWS),
                key=lambda kk: loads[kk])
        assign[w] = k
        loads[k] += counts[w]
        nwin[k] += 1
    # per-core windows sorted by count desc -> slot j balanced across cores
    core_wins = []
    for k in range(NCORES):
        ws = np.where(assign == k)[0]
        ws = ws[np.argsort(-counts[ws], kind="stable")]
        core_wins.append(ws)
    core_wins = np.stack(core_wins)          # [NCORES, WINDOWS] global win id

    mW = [max(1, int(np.ceil(max(counts[core_wins[k][j]]
                                 for k in range(NCORES)) / P)))
          for j in range(WINDOWS)]
    dims = dict(mW=mW, sumM=sum(mW), sumE=sum(mW) * P, mmax=max(mW))

    # bucket edges by window
    eorder = np.argsort(win, kind="stable")
    estart = np.searchsorted(win[eorder], np.arange(NWIN + 1))

    batch_pad = np.full(NODES_PAD, -1.0, np.float32)
    batch_pad[:N_NODES] = batch.astype(np.float32)

    per_core = []
    for k in range(NCORES):
        sidx = np.zeros((16, dims["sumE"] // 16), np.int16)
        didx = np.zeros((16, dims["sumE"] // 16), np.int16)
        dstw = np.full((P, dims["sumM"]), -1.0, bf16)
        mskw = np.zeros((P, dims["sumM"]), bf16)
        bv = np.zeros((WINDOWS, P), np.float32)
        cE = cM = 0
        for j in range(WINDOWS):
            w = core_wins[k][j]
            sl = eorder[estart[w]:estart[w + 1]]
            s, d = src[sl], dst[sl]
            n = len(s)
            L = mW[j] * P
            sidx[:, cE // 16:(cE + L) // 16] = _wrap_idx(
                (s - MID).astype(np.int16), L)
            didx[:, cE // 16:(cE + L) // 16] = _wrap_idx(
                (d - MID).astype(np.int16), L)
            dv = np.full(L, -1.0, np.float32)
            dv[:n] = d - (w << 7)
            mv = np.zeros(L, np.float32)
            mv[:n] = 1.0
            dstw[:, cM:cM + mW[j]] = dv.reshape(mW[j], P).T.astype(bf16)
            mskw[:, cM:cM + mW[j]] = mv.reshape(mW[j], P).T.astype(bf16)
            bv[j] = batch_pad[w << 7:(w + 1) << 7]
            cE += L
            cM += mW[j]
        per_core.append(dict(
            sidx=sidx, didx=didx, dstw=dstw, mskw=mskw,
            bvT=np.ascontiguousarray(bv.T.astype(bf16))))
    return dims, core_wins, per_core


def _prep_weights(x, W1, a_src1, a_dst1, W2, a_src2, a_dst2):
    xT = np.zeros((F_IN_PAD, NODES_PAD), bf16)
    xT[:F_IN, :N_NODES] = x.T.astype(bf16)

    Asrc = np.zeros((H1, HEADS), np.float32)
    Adst = np.zeros((H1, HEADS), np.float32)
    for h in range(HEADS):
        Asrc[h * HID:(h + 1) * HID, h] = a_src1[h]
        Adst[h * HID:(h + 1) * HID, h] = a_dst1[h]
    Waug = np.zeros((F_IN_PAD, 8 + H1), np.float32)
    Waug[:F_IN, 0:4] = W1 @ Asrc
    Waug[:F_IN, 4:8] = W1 @ Adst
    Waug[:F_IN, 8:] = W1
    Waug = Waug.astype(bf16)

    W2aug = np.zeros((H1, 66), np.float32)   # [as2 | h2(64) | ad2]
    W2aug[:, 0] = W2 @ a_src2[0]
    W2aug[:, 1:65] = W2
    W2aug[:, 65] = W2 @ a_dst2[0]
    W2aug = W2aug.astype(bf16)
    return xT, Waug, W2aug


# ======================================================================
# launch 1: dense1 (sharded across cores)
# ======================================================================

def build_dense():
    nc = bacc.Bacc("TRN2", target_bir_lowering=False, debug=False,
                   num_swdge_queues=NQ)

    NC = NBLK_CORE * P  # 6272 nodes per core
    xTc_d = nc.dram_tensor("xTc", [F_IN_PAD, NC], BF16, kind="ExternalInput")
    Waug_d = nc.dram_tensor("Waug", [F_IN_PAD, 264], BF16, kind="ExternalInput")
    T1k_d = nc.dram_tensor("T1k", [NC, 264], BF16, kind="ExternalOutput")

    with tile.TileContext(nc) as tc:
        ctx = contextlib.ExitStack()
        with ctx:
            const = ctx.enter_context(tc.tile_pool(name="const", bufs=1))
            waug_t = const.tile([P, 4, 264], BF16)
            nc.sync.dma_start(waug_t[:], Waug_d[:].rearrange("(ko p) c -> p ko c", p=P))

            CH = 10  # node blocks per xT chunk
            with tc.tile_pool(name="dense", bufs=3) as dpool, \
                 tc.tile_pool(name="dpsum", bufs=4, space="PSUM") as dps:
                for c0 in range(0, NBLK_CORE, CH):
                    nchunk = min(CH, NBLK_CORE - c0) * P
                    xt_t = dpool.tile([P, 4, CH * P], BF16, tag="xt")
                    nc.sync.dma_start(
                        xt_t[:, :, :nchunk],
                        xTc_d[:].rearrange("(ko p) n -> p ko n", p=P)[
                            :, :, c0 * P: c0 * P + nchunk],
                    )
                    for b in range(nchunk // P):
                        ps = dps.tile([P, 264], F32, tag="dps")
                        for ko in range(4):
                            nc.tensor.matmul(
                                ps[:],
                                lhsT=xt_t[:, ko, b * P:(b + 1) * P],
                                rhs=waug_t[:, ko, :],
                                start=(ko == 0),
                                stop=(ko == 3),
                            )
                        t1_t = dpool.tile([P, 264], BF16, tag="t1")
                        nc.scalar.copy(t1_t[:], ps[:])
                        nb = c0 + b
                        nc.sync.dma_start(
                            T1k_d[nb * P:(nb + 1) * P, :], t1_t[:])

    nc.compile()
    return nc


# ======================================================================
# launch 2: layer-1 edge phase + dense2
# ======================================================================

def build_phase_a(dims):
    mW = dims["mW"]
    mmax = dims["mmax"]
    nc = bacc.Bacc("TRN2", target_bir_lowering=False, debug=False,
                   num_swdge_queues=NQ)

    T1_d = nc.dram_tensor("T1", [NODES_PAD, T1_COLS], BF16, kind="ExternalInput")
    T1a_d = nc.dram_tensor("T1a", [NODES_PAD, P], BF16, kind="ExternalInput")
    sidx_d = nc.dram_tensor("sidx", [16, dims["sumE"] // 16], I16, kind="ExternalInput")
    didx_d = nc.dram_tensor("didx", [16, dims["sumE"] // 16], I16, kind="ExternalInput")
    dstw_d = nc.dram_tensor("dstw", [P, dims["sumM"]], BF16, kind="ExternalInput")
    iotaF_d = nc.dram_tensor("iotaF", [1, P], BF16, kind="ExternalInput")
    b1_d = nc.dram_tensor("b1r", [1, H1], F32, kind="ExternalInput")
    W2aug_d = nc.dram_tensor("W2aug", [H1, 66], BF16, kind="ExternalInput")

    T2own_d = nc.dram_tensor("T2own", [OWNPAD, 66], F32, kind="ExternalOutput")

    T1mid = T1_d[MID:, :]
    T1attn = T1a_d[MID:, :]

    with tile.TileContext(nc) as tc:
        ctx = contextlib.ExitStack()
        with ctx:
            const = ctx.enter_context(tc.tile_pool(name="const", bufs=1))
            # index loads FIRST (gathers wait on them)
            iaAll = const.tile([P, dims["sumE"] // 16], I16)
            nc.sync.dma_start(
                iaAll[:], sidx_d[None, :, :].to_broadcast(
                    [8, 16, dims["sumE"] // 16]))
            ibAll = const.tile([P, dims["sumE"] // 16], I16)
            nc.sync.dma_start(
                ibAll[:], didx_d[None, :, :].to_broadcast(
                    [8, 16, dims["sumE"] // 16]))
            dstw = const.tile([P, dims["sumM"]], BF16)
            nc.sync.dma_start(dstw[:], dstw_d[:])
            w2aug_t = const.tile([P, 2, 66], BF16)
            nc.sync.dma_start(w2aug_t[:], W2aug_d[:].rearrange("(ko p) c -> p ko c", p=P))
            iotaF_t = const.tile([P, P], BF16)
            nc.sync.dma_start(iotaF_t[:], iotaF_d[:].to_broadcast([P, P]))
            b1_t = const.tile([P, H1], F32)
            nc.sync.dma_start(b1_t[:], b1_d[:].to_broadcast([P, H1]))
            ident_t = const.tile([P, P], F32)
            make_identity(nc, ident_t[:])

            gpool = ctx.enter_context(tc.tile_pool(name="gat", bufs=5))
            wpool = ctx.enter_context(tc.tile_pool(name="win", bufs=3))
            spool = ctx.enter_context(tc.tile_pool(name="small", bufs=2))
            ps_agg = ctx.enter_context(tc.tile_pool(name="psagg", bufs=2, space="PSUM"))
            ps_z1t = ctx.enter_context(tc.tile_pool(name="psz1t", bufs=2, space="PSUM"))
            ps_h2 = ctx.enter_context(tc.tile_pool(name="psh2", bufs=2, space="PSUM"))

            cE = cM = 0
            for w in range(WINDOWS):
                m = mW[w]
                rows = P

                v_t = gpool.tile([P, mmax, T1_COLS], BF16, tag="v")
                nc.gpsimd.dma_gather(
                    out_ap=v_t[:, 0:m, :], in_ap=T1mid,
                    idxs_ap=iaAll[:, cE // 16:(cE + m * P) // 16],
                    num_idxs=m * P, num_idxs_reg=m * P, elem_size=T1_COLS,
                    single_packet=False, queue_num=(2 * w) % NQ)
                a_t = gpool.tile([P, mmax, P], BF16, tag="a")
                nc.gpsimd.dma_gather(
                    out_ap=a_t[:, 0:m, :], in_ap=T1attn,
                    idxs_ap=ibAll[:, cE // 16:(cE + m * P) // 16],
                    num_idxs=m * P, num_idxs_reg=m * P, elem_size=P,
                    single_packet=False, queue_num=(2 * w + 1) % NQ)

                dcol_t = dstw[:, cM:cM + m]

                # --- S (edge-major indicator); pad slots have dst=-1 so the
                # indicator row is all-zero and padding never contributes ---
                s_t = wpool.tile([P, mmax, P], BF16, tag="s")
                nc.vector.tensor_tensor(
                    s_t[:, :m, :],
                    dcol_t[:, :, None].to_broadcast([P, m, P]),
                    iotaF_t[:, None, :].to_broadcast([P, m, P]),
                    ALU.is_equal)

                # --- ex = exp(lrelu(as + ad)) (padding killed by s_t) ---
                zf = spool.tile([P, mmax, 4], F32, tag="zf")
                nc.vector.tensor_tensor(
                    zf[:, :m, :], v_t[:, :m, 0:4], a_t[:, :m, 4:8], ALU.add)
                zt = spool.tile([P, mmax, 4], F32, tag="zt")
                nc.vector.tensor_scalar_mul(zt[:, :m, :], zf[:, :m, :], NEG_SLOPE)
                nc.vector.tensor_tensor(zt[:, :m, :], zt[:, :m, :], zf[:, :m, :],
                                        ALU.max)

                # --- Vw = [h*ex | ex]: exp lands directly in vw cols 256:260 ---
                vw_t = wpool.tile([P, mmax, 260], BF16, tag="vw")
                nc.scalar.activation(vw_t[:, :m, 256:260], zt[:, :m, :], AF.Exp)
                nc.vector.tensor_tensor(
                    vw_t[:, :m, 0:256].rearrange("p m (h c) -> p m h c", h=HEADS),
                    v_t[:, :m, 8:264].rearrange("p m (h c) -> p m h c", h=HEADS),
                    vw_t[:, :m, 256:260][:, :, :, None].to_broadcast(
                        [P, m, HEADS, HID]),
                    ALU.mult)

                # --- aggregate ---
                pagg = ps_agg.tile([P, 260], F32, tag="psagg")
                for j in range(m):
                    nc.tensor.matmul(
                        pagg[:], lhsT=s_t[:, j, :], rhs=vw_t[:, j, :],
                        start=(j == 0), stop=(j == m - 1))
                # --- out1 = agg / s + b1 ; z1 = relu ---
                sden = spool.tile([P, 4], F32, tag="sden")
                nc.vector.tensor_scalar_add(sden[:], pagg[:, 256:260], EPS)
                nc.vector.reciprocal(sden[:], sden[:])
                z1 = spool.tile([P, H1], F32, tag="z1")
                nc.vector.tensor_tensor(
                    z1[:].rearrange("p (h c) -> p h c", h=HEADS),
                    pagg[:, 0:256].rearrange("p (h c) -> p h c", h=HEADS),
                    sden[:, :, None].to_broadcast([P, HEADS, HID]),
                    ALU.mult)
                nc.vector.tensor_add(z1[:], z1[:], b1_t[:])
                nc.scalar.activation(z1[:], z1[:], AF.Relu)

                # --- dense 2: [as2|h2|ad2] = z1 @ W2aug ---
                z1t = spool.tile([P, 2, P], BF16, tag="z1t")
                for hh in range(2):
                    pzt = ps_z1t.tile([P, P], F32, tag="psz1t")
                    nc.tensor.transpose(
                        pzt[:], z1[:, hh * P:(hh + 1) * P], ident_t[:])
                    nc.scalar.copy(z1t[:, hh, :], pzt[:])
                ph2 = ps_h2.tile([P, 66], F32, tag="psh2")
                for hh in range(2):
                    nc.tensor.matmul(
                        ph2[:], lhsT=z1t[:, hh, :], rhs=w2aug_t[:, hh, :],
                        start=(hh == 0), stop=(hh == 1))
                t2_t = spool.tile([P, 66], F32, tag="t2")
                nc.vector.tensor_copy(t2_t[:], ph2[:])
                nc.sync.dma_start(
                    T2own_d[w * P: w * P + rows, :], t2_t[:rows, :])

                cE += m * P
                cM += m

    nc.compile()
    return nc


# ======================================================================
# launch 3: layer-2 edge phase + pool + logits
# ======================================================================

def build_phase_b(dims):
    mW = dims["mW"]
    mmax = dims["mmax"]
    nc = bacc.Bacc("TRN2", target_bir_lowering=False, debug=False,
                   num_swdge_queues=NQ)

    T2_d = nc.dram_tensor("T2", [NODES_PAD, T2_COLS], BF16, kind="ExternalInput")
    sidx_d = nc.dram_tensor("sidx", [16, dims["sumE"] // 16], I16, kind="ExternalInput")
    didx_d = nc.dram_tensor("didx", [16, dims["sumE"] // 16], I16, kind="ExternalInput")
    dstw_d = nc.dram_tensor("dstw", [P, dims["sumM"]], BF16, kind="ExternalInput")
    iotaF_d = nc.dram_tensor("iotaF", [1, P], BF16, kind="ExternalInput")
    giota_d = nc.dram_tensor("giota", [1, N_GRAPHS], BF16, kind="ExternalInput")
    bvT_d = nc.dram_tensor("bvT", [P, WINDOWS], BF16, kind="ExternalInput")
    b2_d = nc.dram_tensor("b2r", [1, HID], F32, kind="ExternalInput")
    cnt_d = nc.dram_tensor("cnt", [N_GRAPHS, 1], F32, kind="ExternalInput")
    Wl_d = nc.dram_tensor("Wl", [HID, 2], F32, kind="ExternalInput")

    out_d = nc.dram_tensor("partial", [N_GRAPHS, 2], F32, kind="ExternalOutput")

    T2mid = T2_d[MID:, :]

    with tile.TileContext(nc) as tc:
        ctx = contextlib.ExitStack()
        with ctx:
            const = ctx.enter_context(tc.tile_pool(name="const", bufs=1))
            iaAll = const.tile([P, dims["sumE"] // 16], I16)
            nc.sync.dma_start(
                iaAll[:], sidx_d[None, :, :].to_broadcast(
                    [8, 16, dims["sumE"] // 16]))
            ibAll = const.tile([P, dims["sumE"] // 16], I16)
            nc.sync.dma_start(
                ibAll[:], didx_d[None, :, :].to_broadcast(
                    [8, 16, dims["sumE"] // 16]))
            dstw = const.tile([P, dims["sumM"]], BF16)
            nc.sync.dma_start(dstw[:], dstw_d[:])
            bvAll = const.tile([P, WINDOWS], BF16)
            nc.sync.dma_start(bvAll[:], bvT_d[:])
            iotaF_t = const.tile([P, P], BF16)
            nc.sync.dma_start(iotaF_t[:], iotaF_d[:].to_broadcast([P, P]))
            giota_t = const.tile([P, N_GRAPHS], BF16)
            nc.sync.dma_start(giota_t[:], giota_d[:].to_broadcast([P, N_GRAPHS]))
            b2_t = const.tile([P, HID], F32)
            nc.sync.dma_start(b2_t[:], b2_d[:].to_broadcast([P, HID]))
            cnt_t = const.tile([N_GRAPHS, 1], F32)
            nc.sync.dma_start(cnt_t[:], cnt_d[:])
            wl_t = const.tile([P, 2], F32)
            nc.vector.memset(wl_t[:], 0.0)
            nc.sync.dma_start(wl_t[:HID, :], Wl_d[:])
            ident_t = const.tile([P, P], F32)
            make_identity(nc, ident_t[:])
            pts = const.tile([P, N_GRAPHS], F32)
            nc.vector.memset(pts[:], 0.0)

            gpool = ctx.enter_context(tc.tile_pool(name="gat", bufs=5))
            wpool = ctx.enter_context(tc.tile_pool(name="win", bufs=3))
            spool = ctx.enter_context(tc.tile_pool(name="small", bufs=2))
            ps_agg = ctx.enter_context(tc.tile_pool(name="psagg", bufs=2, space="PSUM"))
            ps_pool = ctx.enter_context(tc.tile_pool(name="pspool", bufs=1, space="PSUM"))
            ps_fin = ctx.enter_context(tc.tile_pool(name="psfin", bufs=1, space="PSUM"))

            ppool = ps_pool.tile([N_GRAPHS, HID], F32)

            cE = cM = 0
            for w in range(WINDOWS):
                m = mW[w]

                v_t = gpool.tile([P, mmax, T2_COLS], BF16, tag="v")
                nc.gpsimd.dma_gather(
                    out_ap=v_t[:, 0:m, :], in_ap=T2mid,
                    idxs_ap=iaAll[:, cE // 16:(cE + m * P) // 16],
                    num_idxs=m * P, num_idxs_reg=m * P, elem_size=T2_COLS,
                    single_packet=False, queue_num=(2 * w) % NQ)
                a_t = gpool.tile([P, mmax, T2_COLS], BF16, tag="a")
                nc.gpsimd.dma_gather(
                    out_ap=a_t[:, 0:m, :], in_ap=T2mid,
                    idxs_ap=ibAll[:, cE // 16:(cE + m * P) // 16],
                    num_idxs=m * P, num_idxs_reg=m * P, elem_size=T2_COLS,
                    single_packet=False, queue_num=(2 * w + 1) % NQ)

                dcol_t = dstw[:, cM:cM + m]

                s_t = wpool.tile([P, mmax, P], BF16, tag="s")
                nc.vector.tensor_tensor(
                    s_t[:, :m, :],
                    dcol_t[:, :, None].to_broadcast([P, m, P]),
                    iotaF_t[:, None, :].to_broadcast([P, m, P]),
                    ALU.is_equal)
                zf = spool.tile([P, mmax], F32, tag="zf")
                nc.vector.tensor_tensor(
                    zf[:, :m], v_t[:, :m, 0], a_t[:, :m, 65], ALU.add)
                zt = spool.tile([P, mmax], F32, tag="zt")
                nc.vector.tensor_scalar_mul(zt[:, :m], zf[:, :m], NEG_SLOPE)
                nc.vector.tensor_tensor(zt[:, :m], zt[:, :m], zf[:, :m], ALU.max)

                vw_t = wpool.tile([P, mmax, 65], BF16, tag="vw")
                nc.scalar.activation(vw_t[:, :m, 64:65], zt[:, :m, None], AF.Exp)
                nc.vector.tensor_tensor(
                    vw_t[:, :m, 0:64],
                    v_t[:, :m, 1:65],
                    vw_t[:, :m, 64:65].to_broadcast([P, m, HID]),
                    ALU.mult)

                pagg = ps_agg.tile([P, 65], F32, tag="psagg")
                for j in range(m):
                    nc.tensor.matmul(
                        pagg[:], lhsT=s_t[:, j, :], rhs=vw_t[:, j, :],
                        start=(j == 0), stop=(j == m - 1))
                sden = spool.tile([P, 1], F32, tag="sden")
                nc.vector.tensor_scalar_add(sden[:], pagg[:, 64:65], EPS)
                nc.vector.reciprocal(sden[:], sden[:])
                z2 = spool.tile([P, HID], F32, tag="z2")
                nc.vector.tensor_tensor(
                    z2[:], pagg[:, 0:64], sden[:].to_broadcast([P, HID]),
                    ALU.mult)
                nc.vector.tensor_add(z2[:], z2[:], b2_t[:])
                z2b = spool.tile([P, HID], BF16, tag="z2b")
                nc.scalar.activation(z2b[:], z2[:], AF.Relu)

                pw_t = spool.tile([P, N_GRAPHS], BF16, tag="pw")
                nc.vector.tensor_tensor(
                    pw_t[:], bvAll[:, w, None].to_broadcast([P, N_GRAPHS]),
                    giota_t[:], ALU.is_equal)
                nc.tensor.matmul(
                    ppool[:], lhsT=pw_t[:], rhs=z2b[:],
                    start=(w == 0), stop=(w == WINDOWS - 1))
                cE += m * P
                cM += m

            # pooled partial logits
            crec = spool.tile([N_GRAPHS, 1], F32, tag="crec")
            nc.vector.reciprocal(crec[:], cnt_t[:])
            pooled = spool.tile([N_GRAPHS, HID], F32, tag="pooled")
            nc.vector.tensor_tensor(
                pooled[:], ppool[:], crec[:].to_broadcast([N_GRAPHS, HID]),
                ALU.mult)
            ptp = ps_fin.tile([HID, N_GRAPHS], F32)
            nc.tensor.transpose(ptp[:], pooled[:], ident_t[:N_GRAPHS, :N_GRAPHS])
            nc.vector.tensor_copy(pts[:HID, :], ptp[:])
            plog = ps_fin.tile([N_GRAPHS, 2], F32)
            nc.tensor.matmul(plog[:], lhsT=pts[:], rhs=wl_t[:],
                             start=True, stop=True)
            outs = spool.tile([N_GRAPHS, 2], F32, tag="outs")
            nc.vector.tensor_copy(outs[:], plog[:])
            nc.sync.dma_start(out_d[:], outs[:])

    nc.compile()
    return nc


# ======================================================================
# driver
# ======================================================================

def _run(nc, in_maps, label):
    res = bass_utils.run_bass_kernel_spmd(
        nc, in_maps, core_ids=list(range(NCORES)), trace=TRACE)
    if TRACE:
        LAST_TIMES[label] = res.exec_time_ns
    return res.results


def kernel(x, edge_index, batch, W1, a_src1, a_dst1, b1,
           W2, a_src2, a_dst2, b2, Wl, bl):
    if TRACE:
        try:
            import axon_shim  # noqa: F401
        except ImportError:
            pass

    x = np.asarray(x, np.float32)
    edge_index = np.asarray(edge_index)
    batch = np.asarray(batch)

    key = hashlib.sha1(edge_index.tobytes() + batch.tobytes()).hexdigest()
    if key in _CACHE:
        dims, core_wins, per_core, nc_d, nc_a, nc_b = _CACHE[key]
    else:
        dims, core_wins, per_core = _prep(edge_index, batch)
        nc_d = build_dense()
        nc_a = build_phase_a(dims)
        nc_b = build_phase_b(dims)
        _CACHE[key] = (dims, core_wins, per_core, nc_d, nc_a, nc_b)

    xT, Waug, W2aug = _prep_weights(
        x, np.asarray(W1, np.float32), np.asarray(a_src1, np.float32),
        np.asarray(a_dst1, np.float32), np.asarray(W2, np.float32),
        np.asarray(a_src2, np.float32), np.asarray(a_dst2, np.float32))

    iotaF = np.arange(P, dtype=np.float32).astype(bf16)[None, :]
    giota = np.arange(N_GRAPHS, dtype=np.float32).astype(bf16)[None, :]
    b1r = np.asarray(b1, np.float32)[None, :]
    b2r = np.asarray(b2, np.float32)[None, :]
    cnt = np.maximum(
        np.bincount(np.asarray(batch).astype(np.int64), minlength=N_GRAPHS), 1
    ).astype(np.float32)[:, None]
    Wl32 = np.asarray(Wl, np.float32)
    bl32 = np.asarray(bl, np.float32)

    # ---- launch 1: sharded dense1 ----
    NC = NBLK_CORE * P
    in_maps_d = []
    for k in range(NCORES):
        c0 = k * NC
        in_maps_d.append(dict(xTc=np.ascontiguousarray(xT[:, c0:c0 + NC]),
                              Waug=Waug))
    res_d = _run(nc_d, in_maps_d, "dense")

    T1 = np.zeros((NODES_PAD, T1_COLS), bf16)
    for k in range(NCORES):
        T1[k * NC:(k + 1) * NC, 0:264] = res_d[k]["T1k"]
    T1a = np.zeros((NODES_PAD, P), bf16)
    T1a[:, 0:8] = T1[:, 0:8]

    # ---- launch 2: layer-1 edge phase ----
    in_maps_a = []
    for k in range(NCORES):
        pc = per_core[k]
        in_maps_a.append(dict(
            T1=T1, T1a=T1a, sidx=pc["sidx"], didx=pc["didx"],
            dstw=pc["dstw"],
            iotaF=iotaF, b1r=b1r, W2aug=W2aug,
        ))
    res_a = _run(nc_a, in_maps_a, "phase_a")

    T2 = np.zeros((NODES_PAD, T2_COLS), bf16)
    for k in range(NCORES):
        t2k = res_a[k]["T2own"].astype(bf16)          # [OWNPAD, 66]
        wins = core_wins[k]
        T2[(wins[:, None] << 7) + np.arange(P)[None, :], 0:66] = \
            t2k.reshape(WINDOWS, P, 66)

    # ---- launch 3: layer-2 edge phase + pool ----
    in_maps_b = []
    for k in range(NCORES):
        pc = per_core[k]
        in_maps_b.append(dict(
            T2=T2, sidx=pc["sidx"], didx=pc["didx"],
            dstw=pc["dstw"],
            iotaF=iotaF, giota=giota,
            bvT=pc["bvT"], b2r=b2r, cnt=cnt, Wl=Wl32,
        ))
    res_b = _run(nc_b, in_maps_b, "phase_b")

    out = np.zeros((N_GRAPHS, 2), np.float32)
    for k in range(NCORES):
        out += res_b[k]["partial"]
    out += bl32[None, :]
    return out


# revision 20
# speedup vs baseline: 1.6891x; 1.1318x over previous
"""GAT (2-layer, 4-head + 1-head) + global mean pool + linear head on 8 TRN2 cores.

v2 design (vs baseline): dst windows (392 blocks of 128 nodes) are
load-balanced across cores (LPT) and slot-sorted so the SPMD per-slot
padding is small. Each edge phase uses TWO dma_gathers per window on
4 parallel SWDGE queues (num_swdge_queues=4 -> Q7 core pairs work
concurrently, ~2.5ns/idx vs 8.3ns on one queue):
  g1: per-edge source row   [as|ad|h1|junk] (768B) by src index
  g2: per-edge dst attn row [as|ad|...]     (256B, elem_step=384) by dst
so the per-edge attention terms (ad for layer 1, ad2 for layer 2) come
from DMA instead of per-edge one-hot matmuls on the Tensor engine.
Single node table with int16 indices via midpoint bias (idx = node - 25088).

Launch 1 (dense): T1[as|ad|h1] = x @ Waug per 1/8 node slice; host
  stitches the [50176, 384] bf16 table (cols 264:384 zero).
Launch 2 (layer-1 edge phase): per-window softmax aggregation via
  indicator matmuls + dense2 -> T2own rows [as2|h2|ad2].
Launch 3 (layer-2 edge phase): same pattern on the [50176, 128] T2
  table -> global mean pool partials -> partial [64,2] logits; host
  sums partials + bl.
"""

import contextlib
import hashlib
import os
import numpy as np
import ml_dtypes

import concourse.bass as bass
import concourse.mybir as mybir
import concourse.tile as tile
from concourse import bacc
from concourse import bass_utils
from concourse.masks import make_identity

bf16 = ml_dtypes.bfloat16
F32 = mybir.dt.float32
BF16 = mybir.dt.bfloat16
I16 = mybir.dt.int16
AF = mybir.ActivationFunctionType
ALU = mybir.AluOpType

# ---- problem constants ----
N_NODES = 50000
N_GRAPHS = 64
F_IN = 500
F_IN_PAD = 512
H1 = 256          # heads*hid layer 1
HEADS = 4
HID = 64
NEG_SLOPE = 0.2
NCORES = 8
P = 128
NODES_PAD = 50176                # 392*128
NWIN = NODES_PAD // P            # 392 global windows
WINDOWS = NWIN // NCORES         # 49 slots per core
MID = NODES_PAD // 2             # 25088: int16 idx bias
T1_COLS = 384                    # bf16 row: [as(4) | ad(4) | h1(256) | 0(120)]
T2_COLS = 128                    # bf16 row: [as2(1) | h2(64) | ad2(1) | 0(62)]
OWNPAD = WINDOWS * P             # 6272 own nodes per core
NBLK_CORE = WINDOWS              # dense: 49 node blocks per core
EPS = 1e-16
NQ = 4                           # SWDGE queues

TRACE = bool(int(os.environ.get("KERNEL_TRACE", "0")))
LAST_TIMES = {}

_CACHE = {}


# ======================================================================
# host preprocessing
# ======================================================================

def _wrap_idx(idx, L):
    pad = np.zeros(L, np.int16)
    pad[: len(idx)] = idx
    return pad.reshape(L // 16, 16).T.astype(np.int16)  # [16, L/16]


def _prep(edge_index, batch):
    src = np.concatenate([edge_index[0], np.arange(N_NODES, dtype=np.int64)])
    dst = np.concatenate([edge_index[1], np.arange(N_NODES, dtype=np.int64)])
    src = src.astype(np.int32)
    dst = dst.astype(np.int32)

    win = dst >> 7
    counts = np.bincount(win, minlength=NWIN)

    # LPT assignment of the 392 windows to 8 cores, 49 each
    order = np.argsort(-counts, kind="stable")
    loads = np.zeros(NCORES, np.int64)
    nwin = np.zeros(NCORES, np.int64)
    assign = np.zeros(NWIN, np.int64)
    for w in order:
        k = min((kk for kk in range(NCORES) if nwin[kk] < WINDOWS),
                key=lambda kk: loads[kk])
        assign[w] = k
        loads[k] += counts[w]
        nwin[k] += 1
    # per-core windows sorted by count desc -> slot j balanced across cores
    core_wins = []
    for k in range(NCORES):
        ws = np.where(assign == k)[0]
        ws = ws[np.argsort(-counts[ws], kind="stable")]
        core_wins.append(ws)
    core_wins = np.stack(core_wins)          # [NCORES, WINDOWS] global win id

    mW = [max(1, int(np.ceil(max(counts[core_wins[k][j]]
                                 for k in range(NCORES)) / P)))
          for j in range(WINDOWS)]
    dims = dict(mW=mW, sumM=sum(mW), sumE=sum(mW) * P, mmax=max(mW))

    # bucket edges by window
    eorder = np.argsort(win, kind="stable")
    estart = np.searchsorted(win[eorder], np.arange(NWIN + 1))

    batch_pad = np.full(NODES_PAD, -1.0, np.float32)
    batch_pad[:N_NODES] = batch.astype(np.float32)

    per_core = []
    for k in range(NCORES):
        sidx = np.zeros((16, dims["sumE"] // 16), np.int16)
        dstw = np.full((P, dims["sumM"]), -1.0, bf16)
        strw = np.zeros((P, dims["sumE"]), bf16)
        bv = np.zeros((WINDOWS, P), np.float32)
        cE = cM = 0
        for j in range(WINDOWS):
            w = core_wins[k][j]
            sl = eorder[estart[w]:estart[w + 1]]
            s, d = src[sl], dst[sl]
            n = len(s)
            L = mW[j] * P
            sidx[:, cE // 16:(cE + L) // 16] = _wrap_idx(
                (s - MID).astype(np.int16), L)
            dv = np.full(L, -1.0, np.float32)
            dv[:n] = d - (w << 7)
            dstw[:, cM:cM + mW[j]] = dv.reshape(mW[j], P).T.astype(bf16)
            strw[(d - (w << 7)), cE + np.arange(n)] = 1.0
            bv[j] = batch_pad[w << 7:(w + 1) << 7]
            cE += L
            cM += mW[j]
        per_core.append(dict(
            sidx=sidx, dstw=dstw, strw=strw,
            bvT=np.ascontiguousarray(bv.T.astype(bf16))))
    return dims, core_wins, per_core


def _prep_weights(x, W1, a_src1, a_dst1, W2, a_src2, a_dst2):
    xT = np.zeros((F_IN_PAD, NODES_PAD), bf16)
    xT[:F_IN, :N_NODES] = x.T.astype(bf16)

    Asrc = np.zeros((H1, HEADS), np.float32)
    Adst = np.zeros((H1, HEADS), np.float32)
    for h in range(HEADS):
        Asrc[h * HID:(h + 1) * HID, h] = a_src1[h]
        Adst[h * HID:(h + 1) * HID, h] = a_dst1[h]
    Waug = np.zeros((F_IN_PAD, 8 + H1), np.float32)
    Waug[:F_IN, 0:4] = W1 @ Asrc
    Waug[:F_IN, 4:8] = W1 @ Adst
    Waug[:F_IN, 8:] = W1
    Waug = Waug.astype(bf16)

    W2aug = np.zeros((H1, 66), np.float32)   # [as2 | h2(64) | ad2]
    W2aug[:, 0] = W2 @ a_src2[0]
    W2aug[:, 1:65] = W2
    W2aug[:, 65] = W2 @ a_dst2[0]
    W2aug = W2aug.astype(bf16)
    return xT, Waug, W2aug


# ======================================================================
# launch 1: dense1 (sharded across cores)
# ======================================================================

def build_dense():
    nc = bacc.Bacc("TRN2", target_bir_lowering=False, debug=False,
                   num_swdge_queues=NQ)

    NC = NBLK_CORE * P  # 6272 nodes per core
    xTc_d = nc.dram_tensor("xTc", [F_IN_PAD, NC], BF16, kind="ExternalInput")
    Waug_d = nc.dram_tensor("Waug", [F_IN_PAD, 264], BF16, kind="ExternalInput")
    T1k_d = nc.dram_tensor("T1k", [NC, 264], BF16, kind="ExternalOutput")

    with tile.TileContext(nc) as tc:
        ctx = contextlib.ExitStack()
        with ctx:
            const = ctx.enter_context(tc.tile_pool(name="const", bufs=1))
            waug_t = const.tile([P, 4, 264], BF16)
            nc.sync.dma_start(waug_t[:], Waug_d[:].rearrange("(ko p) c -> p ko c", p=P))

            CH = 10  # node blocks per xT chunk
            with tc.tile_pool(name="dense", bufs=3) as dpool, \
                 tc.tile_pool(name="dpsum", bufs=4, space="PSUM") as dps:
                for c0 in range(0, NBLK_CORE, CH):
                    nchunk = min(CH, NBLK_CORE - c0) * P
                    xt_t = dpool.tile([P, 4, CH * P], BF16, tag="xt")
                    nc.sync.dma_start(
                        xt_t[:, :, :nchunk],
                        xTc_d[:].rearrange("(ko p) n -> p ko n", p=P)[
                            :, :, c0 * P: c0 * P + nchunk],
                    )
                    for b in range(nchunk // P):
                        ps = dps.tile([P, 264], F32, tag="dps")
                        for ko in range(4):
                            nc.tensor.matmul(
                                ps[:],
                                lhsT=xt_t[:, ko, b * P:(b + 1) * P],
                                rhs=waug_t[:, ko, :],
                                start=(ko == 0),
                                stop=(ko == 3),
                            )
                        t1_t = dpool.tile([P, 264], BF16, tag="t1")
                        nc.scalar.copy(t1_t[:], ps[:])
                        nb = c0 + b
                        nc.sync.dma_start(
                            T1k_d[nb * P:(nb + 1) * P, :], t1_t[:])

    nc.compile()
    return nc


# ======================================================================
# launch 2: layer-1 edge phase + dense2
# ======================================================================

def build_phase_a(dims):
    mW = dims["mW"]
    mmax = dims["mmax"]
    nc = bacc.Bacc("TRN2", target_bir_lowering=False, debug=False,
                   num_swdge_queues=NQ)

    T1_d = nc.dram_tensor("T1", [NODES_PAD, T1_COLS], BF16, kind="ExternalInput")
    sidx_d = nc.dram_tensor("sidx", [16, dims["sumE"] // 16], I16, kind="ExternalInput")
    dstw_d = nc.dram_tensor("dstw", [P, dims["sumM"]], BF16, kind="ExternalInput")
    strw_d = nc.dram_tensor("strw", [P, dims["sumE"]], BF16, kind="ExternalInput")
    adown_d = nc.dram_tensor("adown", [OWNPAD, 8], BF16, kind="ExternalInput")
    iotaF_d = nc.dram_tensor("iotaF", [1, P], BF16, kind="ExternalInput")
    b1_d = nc.dram_tensor("b1r", [1, H1], F32, kind="ExternalInput")
    W2aug_d = nc.dram_tensor("W2aug", [H1, 66], BF16, kind="ExternalInput")

    T2own_d = nc.dram_tensor("T2own", [OWNPAD, 66], F32, kind="ExternalOutput")

    T1mid = T1_d[MID:, :]

    with tile.TileContext(nc) as tc:
        ctx = contextlib.ExitStack()
        with ctx:
            const = ctx.enter_context(tc.tile_pool(name="const", bufs=1))
            # index loads FIRST (gathers wait on them)
            iaAll = const.tile([P, dims["sumE"] // 16], I16)
            nc.sync.dma_start(
                iaAll[:], sidx_d[None, :, :].to_broadcast(
                    [8, 16, dims["sumE"] // 16]))
            dstw = const.tile([P, dims["sumM"]], BF16)
            nc.sync.dma_start(dstw[:], dstw_d[:])
            adown = const.tile([P, WINDOWS, 8], BF16)
            nc.sync.dma_start(
                adown[:], adown_d[:].rearrange("(w p) c -> p w c", p=P))
            w2aug_t = const.tile([P, 2, 66], BF16)
            nc.sync.dma_start(w2aug_t[:], W2aug_d[:].rearrange("(ko p) c -> p ko c", p=P))
            iotaF_t = const.tile([P, P], BF16)
            nc.sync.dma_start(iotaF_t[:], iotaF_d[:].to_broadcast([P, P]))
            b1_t = const.tile([P, H1], F32)
            nc.sync.dma_start(b1_t[:], b1_d[:].to_broadcast([P, H1]))
            ident_t = const.tile([P, P], F32)
            make_identity(nc, ident_t[:])

            gpool = ctx.enter_context(tc.tile_pool(name="gat", bufs=5))
            tpool = ctx.enter_context(tc.tile_pool(name="str", bufs=3))
            wpool = ctx.enter_context(tc.tile_pool(name="win", bufs=3))
            spool = ctx.enter_context(tc.tile_pool(name="small", bufs=2))
            ps_agg = ctx.enter_context(tc.tile_pool(name="psagg", bufs=2, space="PSUM"))
            ps_ad1 = ctx.enter_context(tc.tile_pool(name="psad1", bufs=2, space="PSUM"))
            ps_z1t = ctx.enter_context(tc.tile_pool(name="psz1t", bufs=2, space="PSUM"))
            ps_h2 = ctx.enter_context(tc.tile_pool(name="psh2", bufs=2, space="PSUM"))

            cE = cM = 0
            for w in range(WINDOWS):
                m = mW[w]
                rows = P

                v_t = gpool.tile([P, mmax, T1_COLS], BF16, tag="v")
                nc.gpsimd.dma_gather(
                    out_ap=v_t[:, 0:m, :], in_ap=T1mid,
                    idxs_ap=iaAll[:, cE // 16:(cE + m * P) // 16],
                    num_idxs=m * P, num_idxs_reg=m * P, elem_size=T1_COLS,
                    single_packet=False, queue_num=w % NQ)
                str_t = tpool.tile([P, mmax * P], BF16, tag="str")
                nc.sync.dma_start(str_t[:, :m * P], strw_d[:, cE:cE + m * P])

                dcol_t = dstw[:, cM:cM + m]

                # --- S (edge-major indicator); pad slots have dst=-1 so the
                # indicator row is all-zero and padding never contributes ---
                s_t = wpool.tile([P, mmax, P], BF16, tag="s")
                nc.vector.tensor_tensor(
                    s_t[:, :m, :],
                    dcol_t[:, :, None].to_broadcast([P, m, P]),
                    iotaF_t[:, None, :].to_broadcast([P, m, P]),
                    ALU.is_equal)

                # --- ad1 per edge via dst-major indicator matmuls ---
                pad1 = ps_ad1.tile([P, 4 * mmax], F32, tag="psad1")
                for j in range(m):
                    nc.tensor.matmul(
                        pad1[:, j * 4:(j + 1) * 4],
                        lhsT=str_t[:, j * P:(j + 1) * P],
                        rhs=adown[:, w, 4:8],
                        start=True, stop=True)

                # --- ex = exp(lrelu(as + ad)) (padding killed by s_t) ---
                zf = spool.tile([P, mmax, 4], F32, tag="zf")
                nc.vector.tensor_tensor(
                    zf[:, :m, :], v_t[:, :m, 0:4],
                    pad1[:].rearrange("p (j c) -> p j c", c=4)[:, :m, :],
                    ALU.add)
                zt = spool.tile([P, mmax, 4], F32, tag="zt")
                nc.vector.tensor_scalar_mul(zt[:, :m, :], zf[:, :m, :], NEG_SLOPE)
                nc.vector.tensor_tensor(zt[:, :m, :], zt[:, :m, :], zf[:, :m, :],
                                        ALU.max)

                # --- Vw = [h*ex | ex]: exp lands directly in vw cols 256:260 ---
                vw_t = wpool.tile([P, mmax, 260], BF16, tag="vw")
                nc.scalar.activation(vw_t[:, :m, 256:260], zt[:, :m, :], AF.Exp)
                nc.vector.tensor_tensor(
                    vw_t[:, :m, 0:256].rearrange("p m (h c) -> p m h c", h=HEADS),
                    v_t[:, :m, 8:264].rearrange("p m (h c) -> p m h c", h=HEADS),
                    vw_t[:, :m, 256:260][:, :, :, None].to_broadcast(
                        [P, m, HEADS, HID]),
                    ALU.mult)

                # --- aggregate ---
                pagg = ps_agg.tile([P, 260], F32, tag="psagg")
                for j in range(m):
                    nc.tensor.matmul(
                        pagg[:], lhsT=s_t[:, j, :], rhs=vw_t[:, j, :],
                        start=(j == 0), stop=(j == m - 1))
                # --- out1 = agg / s + b1 ; z1 = relu ---
                sden = spool.tile([P, 4], F32, tag="sden")
                nc.vector.tensor_scalar_add(sden[:], pagg[:, 256:260], EPS)
                nc.vector.reciprocal(sden[:], sden[:])
                z1 = spool.tile([P, H1], F32, tag="z1")
                nc.vector.tensor_tensor(
                    z1[:].rearrange("p (h c) -> p h c", h=HEADS),
                    pagg[:, 0:256].rearrange("p (h c) -> p h c", h=HEADS),
                    sden[:, :, None].to_broadcast([P, HEADS, HID]),
                    ALU.mult)
                nc.vector.tensor_add(z1[:], z1[:], b1_t[:])
                nc.scalar.activation(z1[:], z1[:], AF.Relu)

                # --- dense 2: [as2|h2|ad2] = z1 @ W2aug ---
                z1t = spool.tile([P, 2, P], BF16, tag="z1t")
                for hh in range(2):
                    pzt = ps_z1t.tile([P, P], F32, tag="psz1t")
                    nc.tensor.transpose(
                        pzt[:], z1[:, hh * P:(hh + 1) * P], ident_t[:])
                    nc.scalar.copy(z1t[:, hh, :], pzt[:])
                ph2 = ps_h2.tile([P, 66], F32, tag="psh2")
                for hh in range(2):
                    nc.tensor.matmul(
                        ph2[:], lhsT=z1t[:, hh, :], rhs=w2aug_t[:, hh, :],
                        start=(hh == 0), stop=(hh == 1))
                t2_t = spool.tile([P, 66], F32, tag="t2")
                nc.vector.tensor_copy(t2_t[:], ph2[:])
                nc.sync.dma_start(
                    T2own_d[w * P: w * P + rows, :], t2_t[:rows, :])

                cE += m * P
                cM += m

    nc.compile()
    return nc


# ======================================================================
# launch 3: layer-2 edge phase + pool + logits
# ======================================================================

def build_phase_b(dims):
    mW = dims["mW"]
    mmax = dims["mmax"]
    nc = bacc.Bacc("TRN2", target_bir_lowering=False, debug=False,
                   num_swdge_queues=NQ)

    T2_d = nc.dram_tensor("T2", [NODES_PAD, T2_COLS], BF16, kind="ExternalInput")
    sidx_d = nc.dram_tensor("sidx", [16, dims["sumE"] // 16], I16, kind="ExternalInput")
    dstw_d = nc.dram_tensor("dstw", [P, dims["sumM"]], BF16, kind="ExternalInput")
    strw_d = nc.dram_tensor("strw", [P, dims["sumE"]], BF16, kind="ExternalInput")
    adown_d = nc.dram_tensor("adown", [OWNPAD, 8], BF16, kind="ExternalInput")
    iotaF_d = nc.dram_tensor("iotaF", [1, P], BF16, kind="ExternalInput")
    giota_d = nc.dram_tensor("giota", [1, N_GRAPHS], BF16, kind="ExternalInput")
    bvT_d = nc.dram_tensor("bvT", [P, WINDOWS], BF16, kind="ExternalInput")
    b2_d = nc.dram_tensor("b2r", [1, HID], F32, kind="ExternalInput")
    cnt_d = nc.dram_tensor("cnt", [N_GRAPHS, 1], F32, kind="ExternalInput")
    Wl_d = nc.dram_tensor("Wl", [HID, 2], F32, kind="ExternalInput")

    out_d = nc.dram_tensor("partial", [N_GRAPHS, 2], F32, kind="ExternalOutput")

    T2mid = T2_d[MID:, :]

    with tile.TileContext(nc) as tc:
        ctx = contextlib.ExitStack()
        with ctx:
            const = ctx.enter_context(tc.tile_pool(name="const", bufs=1))
            iaAll = const.tile([P, dims["sumE"] // 16], I16)
            nc.sync.dma_start(
                iaAll[:], sidx_d[None, :, :].to_broadcast(
                    [8, 16, dims["sumE"] // 16]))
            dstw = const.tile([P, dims["sumM"]], BF16)
            nc.sync.dma_start(dstw[:], dstw_d[:])
            adown = const.tile([P, WINDOWS, 8], BF16)
            nc.sync.dma_start(
                adown[:], adown_d[:].rearrange("(w p) c -> p w c", p=P))
            bvAll = const.tile([P, WINDOWS], BF16)
            nc.sync.dma_start(bvAll[:], bvT_d[:])
            iotaF_t = const.tile([P, P], BF16)
            nc.sync.dma_start(iotaF_t[:], iotaF_d[:].to_broadcast([P, P]))
            giota_t = const.tile([P, N_GRAPHS], BF16)
            nc.sync.dma_start(giota_t[:], giota_d[:].to_broadcast([P, N_GRAPHS]))
            b2_t = const.tile([P, HID], F32)
            nc.sync.dma_start(b2_t[:], b2_d[:].to_broadcast([P, HID]))
            cnt_t = const.tile([N_GRAPHS, 1], F32)
            nc.sync.dma_start(cnt_t[:], cnt_d[:])
            wl_t = const.tile([P, 2], F32)
            nc.vector.memset(wl_t[:], 0.0)
            nc.sync.dma_start(wl_t[:HID, :], Wl_d[:])
            ident_t = const.tile([P, P], F32)
            make_identity(nc, ident_t[:])
            pts = const.tile([P, N_GRAPHS], F32)
            nc.vector.memset(pts[:], 0.0)

            gpool = ctx.enter_context(tc.tile_pool(name="gat", bufs=5))
            tpool = ctx.enter_context(tc.tile_pool(name="str", bufs=3))
            wpool = ctx.enter_context(tc.tile_pool(name="win", bufs=3))
            spool = ctx.enter_context(tc.tile_pool(name="small", bufs=2))
            ps_agg = ctx.enter_context(tc.tile_pool(name="psagg", bufs=2, space="PSUM"))
            ps_ad2 = ctx.enter_context(tc.tile_pool(name="psad2", bufs=2, space="PSUM"))
            ps_pool = ctx.enter_context(tc.tile_pool(name="pspool", bufs=1, space="PSUM"))
            ps_fin = ctx.enter_context(tc.tile_pool(name="psfin", bufs=1, space="PSUM"))

            ppool = ps_pool.tile([N_GRAPHS, HID], F32)

            cE = cM = 0
            for w in range(WINDOWS):
                m = mW[w]

                v_t = gpool.tile([P, mmax, T2_COLS], BF16, tag="v")
                nc.gpsimd.dma_gather(
                    out_ap=v_t[:, 0:m, :], in_ap=T2mid,
                    idxs_ap=iaAll[:, cE // 16:(cE + m * P) // 16],
                    num_idxs=m * P, num_idxs_reg=m * P, elem_size=T2_COLS,
                    single_packet=False, queue_num=w % NQ)
                str_t = tpool.tile([P, mmax * P], BF16, tag="str")
                nc.sync.dma_start(str_t[:, :m * P], strw_d[:, cE:cE + m * P])

                dcol_t = dstw[:, cM:cM + m]

                s_t = wpool.tile([P, mmax, P], BF16, tag="s")
                nc.vector.tensor_tensor(
                    s_t[:, :m, :],
                    dcol_t[:, :, None].to_broadcast([P, m, P]),
                    iotaF_t[:, None, :].to_broadcast([P, m, P]),
                    ALU.is_equal)

                pad2 = ps_ad2.tile([P, mmax], F32, tag="psad2")
                for j in range(m):
                    nc.tensor.matmul(
                        pad2[:, j:j + 1],
                        lhsT=str_t[:, j * P:(j + 1) * P],
                        rhs=adown[:, w, 0:1],
                        start=True, stop=True)

                zf = spool.tile([P, mmax], F32, tag="zf")
                nc.vector.tensor_tensor(
                    zf[:, :m], v_t[:, :m, 0], pad2[:, :m], ALU.add)
                zt = spool.tile([P, mmax], F32, tag="zt")
                nc.vector.tensor_scalar_mul(zt[:, :m], zf[:, :m], NEG_SLOPE)
                nc.vector.tensor_tensor(zt[:, :m], zt[:, :m], zf[:, :m], ALU.max)

                vw_t = wpool.tile([P, mmax, 65], BF16, tag="vw")
                nc.scalar.activation(vw_t[:, :m, 64:65], zt[:, :m, None], AF.Exp)
                nc.vector.tensor_tensor(
                    vw_t[:, :m, 0:64],
                    v_t[:, :m, 1:65],
                    vw_t[:, :m, 64:65].to_broadcast([P, m, HID]),
                    ALU.mult)

                pagg = ps_agg.tile([P, 65], F32, tag="psagg")
                for j in range(m):
                    nc.tensor.matmul(
                        pagg[:], lhsT=s_t[:, j, :], rhs=vw_t[:, j, :],
                        start=(j == 0), stop=(j == m - 1))
                sden = spool.tile([P, 1], F32, tag="sden")
                nc.vector.tensor_scalar_add(sden[:], pagg[:, 64:65], EPS)
                nc.vector.reciprocal(sden[:], sden[:])
                z2 = spool.tile([P, HID], F32, tag="z2")
                nc.vector.tensor_tensor(
                    z2[:], pagg[:, 0:64], sden[:].to_broadcast([P, HID]),
                    ALU.mult)
                nc.vector.tensor_add(z2[:], z2[:], b2_t[:])
                z2b = spool.tile([P, HID], BF16, tag="z2b")
                nc.scalar.activation(z2b[:], z2[:], AF.Relu)

                pw_t = spool.tile([P, N_GRAPHS], BF16, tag="pw")
                nc.vector.tensor_tensor(
                    pw_t[:], bvAll[:, w, None].to_broadcast([P, N_GRAPHS]),
                    giota_t[:], ALU.is_equal)
                nc.tensor.matmul(
                    ppool[:], lhsT=pw_t[:], rhs=z2b[:],
                    start=(w == 0), stop=(w == WINDOWS - 1))
                cE += m * P
                cM += m

            # pooled partial logits
            crec = spool.tile([N_GRAPHS, 1], F32, tag="crec")
            nc.vector.reciprocal(crec[:], cnt_t[:])
            pooled = spool.tile([N_GRAPHS, HID], F32, tag="pooled")
            nc.vector.tensor_tensor(
                pooled[:], ppool[:], crec[:].to_broadcast([N_GRAPHS, HID]),
                ALU.mult)
            ptp = ps_fin.tile([HID, N_GRAPHS], F32)
            nc.tensor.transpose(ptp[:], pooled[:], ident_t[:N_GRAPHS, :N_GRAPHS])
            nc.vector.tensor_copy(pts[:HID, :], ptp[:])
            plog = ps_fin.tile([N_GRAPHS, 2], F32)
            nc.tensor.matmul(plog[:], lhsT=pts[:], rhs=wl_t[:],
                             start=True, stop=True)
            outs = spool.tile([N_GRAPHS, 2], F32, tag="outs")
            nc.vector.tensor_copy(outs[:], plog[:])
            nc.sync.dma_start(out_d[:], outs[:])

    nc.compile()
    return nc


# ======================================================================
# driver
# ======================================================================

def _run(nc, in_maps, label):
    res = bass_utils.run_bass_kernel_spmd(
        nc, in_maps, core_ids=list(range(NCORES)), trace=TRACE)
    if TRACE:
        LAST_TIMES[label] = res.exec_time_ns
    return res.results


def kernel(x, edge_index, batch, W1, a_src1, a_dst1, b1,
           W2, a_src2, a_dst2, b2, Wl, bl):
    if TRACE:
        try:
            import axon_shim  # noqa: F401
        except ImportError:
            pass

    x = np.asarray(x, np.float32)
    edge_index = np.asarray(edge_index)
    batch = np.asarray(batch)

    key = hashlib.sha1(edge_index.tobytes() + batch.tobytes()).hexdigest()
    if key in _CACHE:
        dims, core_wins, per_core, nc_d, nc_a, nc_b = _CACHE[key]
    else:
        dims, core_wins, per_core = _prep(edge_index, batch)
        nc_d = build_dense()
        nc_a = build_phase_a(dims)
        nc_b = build_phase_b(dims)
        _CACHE[key] = (dims, core_wins, per_core, nc_d, nc_a, nc_b)

    xT, Waug, W2aug = _prep_weights(
        x, np.asarray(W1, np.float32), np.asarray(a_src1, np.float32),
        np.asarray(a_dst1, np.float32), np.asarray(W2, np.float32),
        np.asarray(a_src2, np.float32), np.asarray(a_dst2, np.float32))

    iotaF = np.arange(P, dtype=np.float32).astype(bf16)[None, :]
    giota = np.arange(N_GRAPHS, dtype=np.float32).astype(bf16)[None, :]
    b1r = np.asarray(b1, np.float32)[None, :]
    b2r = np.asarray(b2, np.float32)[None, :]
    cnt = np.maximum(
        np.bincount(np.asarray(batch).astype(np.int64), minlength=N_GRAPHS), 1
    ).astype(np.float32)[:, None]
    Wl32 = np.asarray(Wl, np.float32)
    bl32 = np.asarray(bl, np.float32)

    # ---- launch 1: sharded dense1 ----
    NC = NBLK_CORE * P
    in_maps_d = []
    for k in range(NCORES):
        c0 = k * NC
        in_maps_d.append(dict(xTc=np.ascontiguousarray(xT[:, c0:c0 + NC]),
                              Waug=Waug))
    res_d = _run(nc_d, in_maps_d, "dense")

    T1 = np.zeros((NODES_PAD, T1_COLS), bf16)
    for k in range(NCORES):
        T1[k * NC:(k + 1) * NC, 0:264] = res_d[k]["T1k"]

    # ---- launch 2: layer-1 edge phase ----
    own_rows = (core_wins[:, :, None] << 7) + np.arange(P)[None, None, :]
    in_maps_a = []
    for k in range(NCORES):
        pc = per_core[k]
        adown = np.ascontiguousarray(
            T1[own_rows[k].reshape(-1), 0:8])
        in_maps_a.append(dict(
            T1=T1, sidx=pc["sidx"],
            dstw=pc["dstw"], strw=pc["strw"], adown=adown,
            iotaF=iotaF, b1r=b1r, W2aug=W2aug,
        ))
    res_a = _run(nc_a, in_maps_a, "phase_a")

    T2 = np.zeros((NODES_PAD, T2_COLS), bf16)
    for k in range(NCORES):
        t2k = res_a[k]["T2own"].astype(bf16)          # [OWNPAD, 66]
        wins = core_wins[k]
        T2[(wins[:, None] << 7) + np.arange(P)[None, :], 0:66] = \
            t2k.reshape(WINDOWS, P, 66)

    # ---- launch 3: layer-2 edge phase + pool ----
    in_maps_b = []
    for k in range(NCORES):
        pc = per_core[k]
        adown2 = np.zeros((OWNPAD, 8), bf16)
        adown2[:, 0] = res_a[k]["T2own"][:, 65].astype(bf16)
        in_maps_b.append(dict(
            T2=T2, sidx=pc["sidx"],
            dstw=pc["dstw"], strw=pc["strw"], adown=adown2,
            iotaF=iotaF, giota=giota,
            bvT=pc["bvT"], b2r=b2r, cnt=cnt, Wl=Wl32,
        ))
    res_b = _run(nc_b, in_maps_b, "phase_b")

    out = np.zeros((N_GRAPHS, 2), np.float32)
    for k in range(NCORES):
        out += res_b[k]["partial"]
    out += bl32[None, :]
    return out


# revision 31
# speedup vs baseline: 1.8948x; 1.1218x over previous
"""GAT (2-layer, 4-head + 1-head) + global mean pool + linear head on 8 TRN2 cores.

v2 design (vs baseline): dst windows (392 blocks of 128 nodes) are
load-balanced across cores (LPT) and slot-sorted so the SPMD per-slot
padding is small. Each edge phase uses TWO dma_gathers per window on
4 parallel SWDGE queues (num_swdge_queues=4 -> Q7 core pairs work
concurrently, ~2.5ns/idx vs 8.3ns on one queue):
  g1: per-edge source row   [as|ad|h1|junk] (768B) by src index
  g2: per-edge dst attn row [as|ad|...]     (256B, elem_step=384) by dst
so the per-edge attention terms (ad for layer 1, ad2 for layer 2) come
from DMA instead of per-edge one-hot matmuls on the Tensor engine.
Single node table with int16 indices via midpoint bias (idx = node - 25088).

Launch 1 (dense): T1[as|ad|h1] = x @ Waug per 1/8 node slice; host
  stitches the [50176, 384] bf16 table (cols 264:384 zero).
Launch 2 (layer-1 edge phase): per-window softmax aggregation via
  indicator matmuls + dense2 -> T2own rows [as2|h2|ad2].
Launch 3 (layer-2 edge phase): same pattern on the [50176, 128] T2
  table -> global mean pool partials -> partial [64,2] logits; host
  sums partials + bl.
"""

import contextlib
import hashlib
import os
import numpy as np
import ml_dtypes

import concourse.bass as bass
import concourse.mybir as mybir
import concourse.tile as tile
from concourse import bacc
from concourse import bass_utils
from concourse.masks import make_identity

bf16 = ml_dtypes.bfloat16
F32 = mybir.dt.float32
BF16 = mybir.dt.bfloat16
I16 = mybir.dt.int16
AF = mybir.ActivationFunctionType
ALU = mybir.AluOpType

# ---- problem constants ----
N_NODES = 50000
N_GRAPHS = 64
F_IN = 500
F_IN_PAD = 512
H1 = 256          # heads*hid layer 1
HEADS = 4
HID = 64
NEG_SLOPE = 0.2
NCORES = 8
P = 128
NODES_PAD = 50176                # 392*128
NWIN = NODES_PAD // P            # 392 global windows
WINDOWS = NWIN // NCORES         # 49 slots per core
MID = NODES_PAD // 2             # 25088: int16 idx bias
T1_COLS = 384                    # bf16 row: [as(4) | ad(4) | h1(256) | 0(120)]
T2_COLS = 128                    # bf16 row: [as2(1) | h2(64) | ad2(1) | 0(62)]
OWNPAD = WINDOWS * P             # 6272 own nodes per core
NBLK_CORE = WINDOWS              # dense: 49 node blocks per core
EPS = 1e-16
NQ = 4                           # SWDGE queues

TRACE = bool(int(os.environ.get("KERNEL_TRACE", "0")))
LAST_TIMES = {}

_CACHE = {}


# ======================================================================
# host preprocessing
# ======================================================================

def _wrap_idx(idx, L):
    pad = np.zeros(L, np.int16)
    pad[: len(idx)] = idx
    return pad.reshape(L // 16, 16).T.astype(np.int16)  # [16, L/16]


def _prep(edge_index, batch):
    src = np.concatenate([edge_index[0], np.arange(N_NODES, dtype=np.int64)])
    dst = np.concatenate([edge_index[1], np.arange(N_NODES, dtype=np.int64)])
    src = src.astype(np.int32)
    dst = dst.astype(np.int32)

    win = dst >> 7
    counts = np.bincount(win, minlength=NWIN)

    # LPT assignment of the 392 windows to 8 cores, 49 each
    order = np.argsort(-counts, kind="stable")
    loads = np.zeros(NCORES, np.int64)
    nwin = np.zeros(NCORES, np.int64)
    assign = np.zeros(NWIN, np.int64)
    for w in order:
        k = min((kk for kk in range(NCORES) if nwin[kk] < WINDOWS),
                key=lambda kk: loads[kk])
        assign[w] = k
        loads[k] += counts[w]
        nwin[k] += 1
    # per-core windows sorted by count desc -> slot j balanced across cores
    core_wins = []
    for k in range(NCORES):
        ws = np.where(assign == k)[0]
        ws = ws[np.argsort(-counts[ws], kind="stable")]
        core_wins.append(ws)
    core_wins = np.stack(core_wins)          # [NCORES, WINDOWS] global win id

    mW = [max(1, int(np.ceil(max(counts[core_wins[k][j]]
                                 for k in range(NCORES)) / P)))
          for j in range(WINDOWS)]
    dims = dict(mW=mW, sumM=sum(mW), sumE=sum(mW) * P, mmax=max(mW))

    # bucket edges by window
    eorder = np.argsort(win, kind="stable")
    estart = np.searchsorted(win[eorder], np.arange(NWIN + 1))

    batch_pad = np.full(NODES_PAD, -1.0, np.float32)
    batch_pad[:N_NODES] = batch.astype(np.float32)

    per_core = []
    for k in range(NCORES):
        sidx = np.zeros((16, dims["sumE"] // 16), np.int16)
        dstw = np.full((P, dims["sumM"]), -1.0, bf16)
        strw = np.zeros((P, dims["sumE"]), bf16)
        bv = np.zeros((WINDOWS, P), np.float32)
        cE = cM = 0
        for j in range(WINDOWS):
            w = core_wins[k][j]
            sl = eorder[estart[w]:estart[w + 1]]
            s, d = src[sl], dst[sl]
            n = len(s)
            L = mW[j] * P
            sidx[:, cE // 16:(cE + L) // 16] = _wrap_idx(
                (s - MID).astype(np.int16), L)
            dv = np.full(L, -1.0, np.float32)
            dv[:n] = d - (w << 7)
            dstw[:, cM:cM + mW[j]] = dv.reshape(mW[j], P).T.astype(bf16)
            strw[(d - (w << 7)), cE + np.arange(n)] = 1.0
            bv[j] = batch_pad[w << 7:(w + 1) << 7]
            cE += L
            cM += mW[j]
        pwb = (bv[:, :, None] ==
               np.arange(N_GRAPHS, dtype=np.float32)[None, None, :])
        per_core.append(dict(
            sidx=sidx, dstw=dstw, strw=strw,
            pwb=np.ascontiguousarray(
                pwb.reshape(OWNPAD, N_GRAPHS).astype(bf16))))
    return dims, core_wins, per_core


def _prep_weights(x, W1, a_src1, a_dst1, W2, a_src2, a_dst2):
    xT = np.zeros((F_IN_PAD, NODES_PAD), bf16)
    xT[:F_IN, :N_NODES] = x.T.astype(bf16)

    Asrc = np.zeros((H1, HEADS), np.float32)
    Adst = np.zeros((H1, HEADS), np.float32)
    for h in range(HEADS):
        Asrc[h * HID:(h + 1) * HID, h] = a_src1[h]
        Adst[h * HID:(h + 1) * HID, h] = a_dst1[h]
    Waug = np.zeros((F_IN_PAD, 8 + H1), np.float32)
    Waug[:F_IN, 0:4] = W1 @ Asrc
    Waug[:F_IN, 4:8] = W1 @ Adst
    Waug[:F_IN, 8:] = W1
    Waug = Waug.astype(bf16)

    W2aug = np.zeros((H1, 66), np.float32)   # [as2 | h2(64) | ad2]
    W2aug[:, 0] = W2 @ a_src2[0]
    W2aug[:, 1:65] = W2
    W2aug[:, 65] = W2 @ a_dst2[0]
    W2aug = W2aug.astype(bf16)
    return xT, Waug, W2aug


# ======================================================================
# launch 1: dense1 (sharded across cores)
# ======================================================================

def build_dense():
    nc = bacc.Bacc("TRN2", target_bir_lowering=False, debug=False,
                   num_swdge_queues=NQ)

    NC = NBLK_CORE * P  # 6272 nodes per core
    xTc_d = nc.dram_tensor("xTc", [F_IN_PAD, NC], BF16, kind="ExternalInput")
    Waug_d = nc.dram_tensor("Waug", [F_IN_PAD, 264], BF16, kind="ExternalInput")
    T1k_d = nc.dram_tensor("T1k", [NC, 264], BF16, kind="ExternalOutput")

    with tile.TileContext(nc) as tc:
        ctx = contextlib.ExitStack()
        with ctx:
            const = ctx.enter_context(tc.tile_pool(name="const", bufs=1))
            waug_t = const.tile([P, 4, 264], BF16)
            nc.sync.dma_start(waug_t[:], Waug_d[:].rearrange("(ko p) c -> p ko c", p=P))

            CH = 10  # node blocks per xT chunk
            with tc.tile_pool(name="dense", bufs=3) as dpool, \
                 tc.tile_pool(name="dpsum", bufs=4, space="PSUM") as dps:
                for c0 in range(0, NBLK_CORE, CH):
                    nchunk = min(CH, NBLK_CORE - c0) * P
                    xt_t = dpool.tile([P, 4, CH * P], BF16, tag="xt")
                    nc.sync.dma_start(
                        xt_t[:, :, :nchunk],
                        xTc_d[:].rearrange("(ko p) n -> p ko n", p=P)[
                            :, :, c0 * P: c0 * P + nchunk],
                    )
                    for b in range(nchunk // P):
                        ps = dps.tile([P, 264], F32, tag="dps")
                        for ko in range(4):
                            nc.tensor.matmul(
                                ps[:],
                                lhsT=xt_t[:, ko, b * P:(b + 1) * P],
                                rhs=waug_t[:, ko, :],
                                start=(ko == 0),
                                stop=(ko == 3),
                            )
                        t1_t = dpool.tile([P, 264], BF16, tag="t1")
                        nc.scalar.copy(t1_t[:], ps[:])
                        nb = c0 + b
                        nc.sync.dma_start(
                            T1k_d[nb * P:(nb + 1) * P, :], t1_t[:])

    nc.compile()
    return nc


# ======================================================================
# launch 2: layer-1 edge phase + dense2
# ======================================================================

def build_phase_a(dims):
    mW = dims["mW"]
    mmax = dims["mmax"]
    nc = bacc.Bacc("TRN2", target_bir_lowering=False, debug=False,
                   num_swdge_queues=NQ)

    T1_d = nc.dram_tensor("T1", [NODES_PAD, T1_COLS], BF16, kind="ExternalInput")
    sidx_d = nc.dram_tensor("sidx", [16, dims["sumE"] // 16], I16, kind="ExternalInput")
    dstw_d = nc.dram_tensor("dstw", [P, dims["sumM"]], BF16, kind="ExternalInput")
    strw_d = nc.dram_tensor("strw", [P, dims["sumE"]], BF16, kind="ExternalInput")
    adown_d = nc.dram_tensor("adown", [OWNPAD, 8], BF16, kind="ExternalInput")
    iotaF_d = nc.dram_tensor("iotaF", [1, P], BF16, kind="ExternalInput")
    b1_d = nc.dram_tensor("b1r", [1, H1], F32, kind="ExternalInput")
    W2aug_d = nc.dram_tensor("W2aug", [H1, 66], BF16, kind="ExternalInput")

    T2own_d = nc.dram_tensor("T2own", [OWNPAD, 66], F32, kind="ExternalOutput")

    T1mid = T1_d[MID:, :]

    with tile.TileContext(nc) as tc:
        ctx = contextlib.ExitStack()
        with ctx:
            const = ctx.enter_context(tc.tile_pool(name="const", bufs=1))
            # index loads FIRST (gathers wait on them)
            iaAll = const.tile([P, dims["sumE"] // 16], I16)
            nc.sync.dma_start(
                iaAll[:], sidx_d[None, :, :].to_broadcast(
                    [8, 16, dims["sumE"] // 16]))
            dstw = const.tile([P, dims["sumM"]], BF16)
            nc.sync.dma_start(dstw[:], dstw_d[:])
            adown = const.tile([P, WINDOWS, 8], BF16)
            nc.sync.dma_start(
                adown[:], adown_d[:].rearrange("(w p) c -> p w c", p=P))
            w2aug_t = const.tile([P, 2, 66], BF16)
            nc.sync.dma_start(w2aug_t[:], W2aug_d[:].rearrange("(ko p) c -> p ko c", p=P))
            iotaF_t = const.tile([P, P], BF16)
            nc.sync.dma_start(iotaF_t[:], iotaF_d[:].to_broadcast([P, P]))
            b1_t = const.tile([P, H1], F32)
            nc.sync.dma_start(b1_t[:], b1_d[:].to_broadcast([P, H1]))
            ident_t = const.tile([P, P], F32)
            make_identity(nc, ident_t[:])
            eps_t = const.tile([P, 1], F32)
            nc.vector.memset(eps_t[:], EPS)

            gpool = ctx.enter_context(tc.tile_pool(name="gat", bufs=7))
            tpool = ctx.enter_context(tc.tile_pool(name="str", bufs=3))
            wpool = ctx.enter_context(tc.tile_pool(name="win", bufs=3))
            spool = ctx.enter_context(tc.tile_pool(name="small", bufs=2))
            ps_agg = ctx.enter_context(tc.tile_pool(name="psagg", bufs=2, space="PSUM"))
            ps_ad1 = ctx.enter_context(tc.tile_pool(name="psad1", bufs=2, space="PSUM"))
            ps_z1t = ctx.enter_context(tc.tile_pool(name="psz1t", bufs=2, space="PSUM"))
            ps_h2 = ctx.enter_context(tc.tile_pool(name="psh2", bufs=2, space="PSUM"))

            cE = cM = 0
            for w in range(WINDOWS):
                m = mW[w]
                rows = P

                v_t = gpool.tile([P, mmax, T1_COLS], BF16, tag="v")
                nc.gpsimd.dma_gather(
                    out_ap=v_t[:, 0:m, :], in_ap=T1mid,
                    idxs_ap=iaAll[:, cE // 16:(cE + m * P) // 16],
                    num_idxs=m * P, num_idxs_reg=m * P, elem_size=T1_COLS,
                    single_packet=False, queue_num=w % NQ)
                str_t = tpool.tile([P, mmax * P], BF16, tag="str")
                nc.sync.dma_start(str_t[:, :m * P], strw_d[:, cE:cE + m * P])

                dcol_t = dstw[:, cM:cM + m]

                # --- S (edge-major indicator); pad slots have dst=-1 so the
                # indicator row is all-zero and padding never contributes ---
                s_t = wpool.tile([P, mmax, P], BF16, tag="s")
                nc.vector.tensor_tensor(
                    s_t[:, :m, :],
                    dcol_t[:, :, None].to_broadcast([P, m, P]),
                    iotaF_t[:, None, :].to_broadcast([P, m, P]),
                    ALU.is_equal)

                # --- ad1 per edge via dst-major indicator matmuls ---
                pad1 = ps_ad1.tile([P, 4 * mmax], F32, tag="psad1")
                for j in range(m):
                    nc.tensor.matmul(
                        pad1[:, j * 4:(j + 1) * 4],
                        lhsT=str_t[:, j * P:(j + 1) * P],
                        rhs=adown[:, w, 4:8],
                        start=True, stop=True)

                # --- ex = exp(lrelu(as + ad)) (padding killed by s_t) ---
                zf = spool.tile([P, mmax, 4], F32, tag="zf")
                nc.vector.tensor_tensor(
                    zf[:, :m, :], v_t[:, :m, 0:4],
                    pad1[:].rearrange("p (j c) -> p j c", c=4)[:, :m, :],
                    ALU.add)
                zt = spool.tile([P, mmax, 4], F32, tag="zt")
                nc.vector.tensor_scalar_mul(zt[:, :m, :], zf[:, :m, :], NEG_SLOPE)
                nc.vector.tensor_tensor(zt[:, :m, :], zt[:, :m, :], zf[:, :m, :],
                                        ALU.max)

                # --- Vw = [h*ex | ex]: exp lands directly in vw cols 256:260 ---
                vw_t = wpool.tile([P, mmax, 260], BF16, tag="vw")
                nc.scalar.activation(vw_t[:, :m, 256:260], zt[:, :m, :], AF.Exp)
                nc.vector.tensor_tensor(
                    vw_t[:, :m, 0:256].rearrange("p m (h c) -> p m h c", h=HEADS),
                    v_t[:, :m, 8:264].rearrange("p m (h c) -> p m h c", h=HEADS),
                    vw_t[:, :m, 256:260][:, :, :, None].to_broadcast(
                        [P, m, HEADS, HID]),
                    ALU.mult)

                # --- aggregate ---
                pagg = ps_agg.tile([P, 260], F32, tag="psagg")
                for j in range(m):
                    nc.tensor.matmul(
                        pagg[:], lhsT=s_t[:, j, :], rhs=vw_t[:, j, :],
                        start=(j == 0), stop=(j == m - 1))
                # --- out1 = agg / den + b1 ; z1 = relu ---
                denS = spool.tile([P, 4], F32, tag="denS")
                nc.scalar.activation(denS[:], pagg[:, 256:260], AF.Identity,
                                     bias=eps_t[:])
                nc.vector.reciprocal(denS[:], denS[:])
                z1 = spool.tile([P, H1], F32, tag="z1")
                nc.vector.tensor_tensor(
                    z1[:].rearrange("p (h c) -> p h c", h=HEADS),
                    pagg[:, 0:256].rearrange("p (h c) -> p h c", h=HEADS),
                    denS[:, :, None].to_broadcast([P, HEADS, HID]),
                    ALU.mult)
                nc.vector.tensor_add(z1[:], z1[:], b1_t[:])
                nc.scalar.activation(z1[:], z1[:], AF.Relu)

                # --- dense 2: [as2|h2|ad2] = z1 @ W2aug ---
                z1t = spool.tile([P, 2, P], BF16, tag="z1t")
                for hh in range(2):
                    pzt = ps_z1t.tile([P, P], F32, tag="psz1t")
                    nc.tensor.transpose(
                        pzt[:], z1[:, hh * P:(hh + 1) * P], ident_t[:])
                    nc.scalar.copy(z1t[:, hh, :], pzt[:])
                ph2 = ps_h2.tile([P, 66], F32, tag="psh2")
                for hh in range(2):
                    nc.tensor.matmul(
                        ph2[:], lhsT=z1t[:, hh, :], rhs=w2aug_t[:, hh, :],
                        start=(hh == 0), stop=(hh == 1))
                t2_t = spool.tile([P, 66], F32, tag="t2")
                nc.scalar.copy(t2_t[:], ph2[:])
                nc.sync.dma_start(
                    T2own_d[w * P: w * P + rows, :], t2_t[:rows, :])

                cE += m * P
                cM += m

    nc.compile()
    return nc


# ======================================================================
# launch 3: layer-2 edge phase + pool + logits
# ======================================================================

def build_phase_b(dims):
    mW = dims["mW"]
    mmax = dims["mmax"]
    nc = bacc.Bacc("TRN2", target_bir_lowering=False, debug=False,
                   num_swdge_queues=NQ)

    T2_d = nc.dram_tensor("T2", [NODES_PAD, T2_COLS], BF16, kind="ExternalInput")
    sidx_d = nc.dram_tensor("sidx", [16, dims["sumE"] // 16], I16, kind="ExternalInput")
    dstw_d = nc.dram_tensor("dstw", [P, dims["sumM"]], BF16, kind="ExternalInput")
    strw_d = nc.dram_tensor("strw", [P, dims["sumE"]], BF16, kind="ExternalInput")
    adown_d = nc.dram_tensor("adown", [OWNPAD, 8], BF16, kind="ExternalInput")
    iotaF_d = nc.dram_tensor("iotaF", [1, P], BF16, kind="ExternalInput")
    pwb_d = nc.dram_tensor("pwb", [OWNPAD, N_GRAPHS], BF16, kind="ExternalInput")
    b2_d = nc.dram_tensor("b2r", [1, HID], F32, kind="ExternalInput")
    cnt_d = nc.dram_tensor("cnt", [N_GRAPHS, 1], F32, kind="ExternalInput")
    Wl_d = nc.dram_tensor("Wl", [HID, 2], F32, kind="ExternalInput")

    out_d = nc.dram_tensor("partial", [N_GRAPHS, 2], F32, kind="ExternalOutput")

    T2mid = T2_d[MID:, :]

    with tile.TileContext(nc) as tc:
        ctx = contextlib.ExitStack()
        with ctx:
            const = ctx.enter_context(tc.tile_pool(name="const", bufs=1))
            iaAll = const.tile([P, dims["sumE"] // 16], I16)
            nc.sync.dma_start(
                iaAll[:], sidx_d[None, :, :].to_broadcast(
                    [8, 16, dims["sumE"] // 16]))
            dstw = const.tile([P, dims["sumM"]], BF16)
            nc.sync.dma_start(dstw[:], dstw_d[:])
            adown = const.tile([P, WINDOWS, 8], BF16)
            nc.sync.dma_start(
                adown[:], adown_d[:].rearrange("(w p) c -> p w c", p=P))
            pwall = const.tile([P, WINDOWS, N_GRAPHS], BF16)
            nc.sync.dma_start(
                pwall[:], pwb_d[:].rearrange("(w p) g -> p w g", p=P))
            iotaF_t = const.tile([P, P], BF16)
            nc.sync.dma_start(iotaF_t[:], iotaF_d[:].to_broadcast([P, P]))
            b2_t = const.tile([P, HID], F32)
            nc.sync.dma_start(b2_t[:], b2_d[:].to_broadcast([P, HID]))
            cnt_t = const.tile([N_GRAPHS, 1], F32)
            nc.sync.dma_start(cnt_t[:], cnt_d[:])
            wl_t = const.tile([P, 2], F32)
            nc.vector.memset(wl_t[:], 0.0)
            nc.sync.dma_start(wl_t[:HID, :], Wl_d[:])
            ident_t = const.tile([P, P], F32)
            make_identity(nc, ident_t[:])
            pts = const.tile([P, N_GRAPHS], F32)
            nc.vector.memset(pts[:], 0.0)
            eps_t = const.tile([P, 1], F32)
            nc.vector.memset(eps_t[:], EPS)

            gpool = ctx.enter_context(tc.tile_pool(name="gat", bufs=7))
            tpool = ctx.enter_context(tc.tile_pool(name="str", bufs=3))
            wpool = ctx.enter_context(tc.tile_pool(name="win", bufs=3))
            spool = ctx.enter_context(tc.tile_pool(name="small", bufs=2))
            ps_agg = ctx.enter_context(tc.tile_pool(name="psagg", bufs=2, space="PSUM"))
            ps_ad2 = ctx.enter_context(tc.tile_pool(name="psad2", bufs=2, space="PSUM"))
            ps_pool = ctx.enter_context(tc.tile_pool(name="pspool", bufs=1, space="PSUM"))
            ps_fin = ctx.enter_context(tc.tile_pool(name="psfin", bufs=1, space="PSUM"))

            ppool = ps_pool.tile([N_GRAPHS, HID], F32)

            cE = cM = 0
            for w in range(WINDOWS):
                m = mW[w]

                v_t = gpool.tile([P, mmax, T2_COLS], BF16, tag="v")
                nc.gpsimd.dma_gather(
                    out_ap=v_t[:, 0:m, :], in_ap=T2mid,
                    idxs_ap=iaAll[:, cE // 16:(cE + m * P) // 16],
                    num_idxs=m * P, num_idxs_reg=m * P, elem_size=T2_COLS,
                    single_packet=False, queue_num=w % NQ)
                str_t = tpool.tile([P, mmax * P], BF16, tag="str")
                nc.sync.dma_start(str_t[:, :m * P], strw_d[:, cE:cE + m * P])

                dcol_t = dstw[:, cM:cM + m]

                s_t = wpool.tile([P, mmax, P], BF16, tag="s")
                nc.vector.tensor_tensor(
                    s_t[:, :m, :],
                    dcol_t[:, :, None].to_broadcast([P, m, P]),
                    iotaF_t[:, None, :].to_broadcast([P, m, P]),
                    ALU.is_equal)

                pad2 = ps_ad2.tile([P, mmax], F32, tag="psad2")
                for j in range(m):
                    nc.tensor.matmul(
                        pad2[:, j:j + 1],
                        lhsT=str_t[:, j * P:(j + 1) * P],
                        rhs=adown[:, w, 0:1],
                        start=True, stop=True)

                zf = spool.tile([P, mmax], F32, tag="zf")
                nc.vector.tensor_tensor(
                    zf[:, :m], v_t[:, :m, 0], pad2[:, :m], ALU.add)
                zt = spool.tile([P, mmax], F32, tag="zt")
                nc.vector.tensor_scalar_mul(zt[:, :m], zf[:, :m], NEG_SLOPE)
                nc.vector.tensor_tensor(zt[:, :m], zt[:, :m], zf[:, :m], ALU.max)

                vw_t = wpool.tile([P, mmax, 65], BF16, tag="vw")
                nc.scalar.activation(vw_t[:, :m, 64:65], zt[:, :m, None], AF.Exp)
                nc.vector.tensor_tensor(
                    vw_t[:, :m, 0:64],
                    v_t[:, :m, 1:65],
                    vw_t[:, :m, 64:65].to_broadcast([P, m, HID]),
                    ALU.mult)

                pagg = ps_agg.tile([P, 65], F32, tag="psagg")
                for j in range(m):
                    nc.tensor.matmul(
                        pagg[:], lhsT=s_t[:, j, :], rhs=vw_t[:, j, :],
                        start=(j == 0), stop=(j == m - 1))
                denS = spool.tile([P, 1], F32, tag="denS")
                nc.scalar.activation(denS[:], pagg[:, 64:65], AF.Identity,
                                     bias=eps_t[:])
                nc.vector.reciprocal(denS[:], denS[:])
                z2 = spool.tile([P, HID], F32, tag="z2")
                nc.vector.tensor_tensor(
                    z2[:], pagg[:, 0:64], denS[:].to_broadcast([P, HID]),
                    ALU.mult)
                nc.vector.tensor_add(z2[:], z2[:], b2_t[:])
                z2b = spool.tile([P, HID], BF16, tag="z2b")
                nc.scalar.activation(z2b[:], z2[:], AF.Relu)

                nc.tensor.matmul(
                    ppool[:], lhsT=pwall[:, w, :], rhs=z2b[:],
                    start=(w == 0), stop=(w == WINDOWS - 1))
                cE += m * P
                cM += m

            # pooled partial logits
            crec = spool.tile([N_GRAPHS, 1], F32, tag="crec")
            nc.vector.reciprocal(crec[:], cnt_t[:])
            pooled = spool.tile([N_GRAPHS, HID], F32, tag="pooled")
            nc.vector.tensor_tensor(
                pooled[:], ppool[:], crec[:].to_broadcast([N_GRAPHS, HID]),
                ALU.mult)
            ptp = ps_fin.tile([HID, N_GRAPHS], F32)
            nc.tensor.transpose(ptp[:], pooled[:], ident_t[:N_GRAPHS, :N_GRAPHS])
            nc.vector.tensor_copy(pts[:HID, :], ptp[:])
            plog = ps_fin.tile([N_GRAPHS, 2], F32)
            nc.tensor.matmul(plog[:], lhsT=pts[:], rhs=wl_t[:],
                             start=True, stop=True)
            outs = spool.tile([N_GRAPHS, 2], F32, tag="outs")
            nc.vector.tensor_copy(outs[:], plog[:])
            nc.sync.dma_start(out_d[:], outs[:])

    nc.compile()
    return nc


# ======================================================================
# driver
# ======================================================================

def _run(nc, in_maps, label):
    res = bass_utils.run_bass_kernel_spmd(
        nc, in_maps, core_ids=list(range(NCORES)), trace=TRACE)
    if TRACE:
        LAST_TIMES[label] = res.exec_time_ns
    return res.results


def kernel(x, edge_index, batch, W1, a_src1, a_dst1, b1,
           W2, a_src2, a_dst2, b2, Wl, bl):
    if TRACE:
        try:
            import axon_shim  # noqa: F401
        except ImportError:
            pass

    x = np.asarray(x, np.float32)
    edge_index = np.asarray(edge_index)
    batch = np.asarray(batch)

    key = hashlib.sha1(edge_index.tobytes() + batch.tobytes()).hexdigest()
    if key in _CACHE:
        dims, core_wins, per_core, nc_d, nc_a, nc_b = _CACHE[key]
    else:
        dims, core_wins, per_core = _prep(edge_index, batch)
        nc_d = build_dense()
        nc_a = build_phase_a(dims)
        nc_b = build_phase_b(dims)
        _CACHE[key] = (dims, core_wins, per_core, nc_d, nc_a, nc_b)

    xT, Waug, W2aug = _prep_weights(
        x, np.asarray(W1, np.float32), np.asarray(a_src1, np.float32),
        np.asarray(a_dst1, np.float32), np.asarray(W2, np.float32),
        np.asarray(a_src2, np.float32), np.asarray(a_dst2, np.float32))

    iotaF = np.arange(P, dtype=np.float32).astype(bf16)[None, :]
    b1r = np.asarray(b1, np.float32)[None, :]
    b2r = np.asarray(b2, np.float32)[None, :]
    cnt = np.maximum(
        np.bincount(np.asarray(batch).astype(np.int64), minlength=N_GRAPHS), 1
    ).astype(np.float32)[:, None]
    Wl32 = np.asarray(Wl, np.float32)
    bl32 = np.asarray(bl, np.float32)

    # ---- launch 1: sharded dense1 ----
    NC = NBLK_CORE * P
    in_maps_d = []
    for k in range(NCORES):
        c0 = k * NC
        in_maps_d.append(dict(xTc=np.ascontiguousarray(xT[:, c0:c0 + NC]),
                              Waug=Waug))
    res_d = _run(nc_d, in_maps_d, "dense")

    T1 = np.zeros((NODES_PAD, T1_COLS), bf16)
    for k in range(NCORES):
        T1[k * NC:(k + 1) * NC, 0:264] = res_d[k]["T1k"]

    # ---- launch 2: layer-1 edge phase ----
    own_rows = (core_wins[:, :, None] << 7) + np.arange(P)[None, None, :]
    in_maps_a = []
    for k in range(NCORES):
        pc = per_core[k]
        adown = np.ascontiguousarray(
            T1[own_rows[k].reshape(-1), 0:8])
        in_maps_a.append(dict(
            T1=T1, sidx=pc["sidx"],
            dstw=pc["dstw"], strw=pc["strw"], adown=adown,
            iotaF=iotaF, b1r=b1r, W2aug=W2aug,
        ))
    res_a = _run(nc_a, in_maps_a, "phase_a")

    T2 = np.zeros((NODES_PAD, T2_COLS), bf16)
    for k in range(NCORES):
        t2k = res_a[k]["T2own"].astype(bf16)          # [OWNPAD, 66]
        wins = core_wins[k]
        T2[(wins[:, None] << 7) + np.arange(P)[None, :], 0:66] = \
            t2k.reshape(WINDOWS, P, 66)

    # ---- launch 3: layer-2 edge phase + pool ----
    in_maps_b = []
    for k in range(NCORES):
        pc = per_core[k]
        adown2 = np.zeros((OWNPAD, 8), bf16)
        adown2[:, 0] = res_a[k]["T2own"][:, 65].astype(bf16)
        in_maps_b.append(dict(
            T2=T2, sidx=pc["sidx"],
            dstw=pc["dstw"], strw=pc["strw"], adown=adown2,
            iotaF=iotaF, pwb=pc["pwb"], b2r=b2r, cnt=cnt, Wl=Wl32,
        ))
    res_b = _run(nc_b, in_maps_b, "phase_b")

    out = np.zeros((N_GRAPHS, 2), np.float32)
    for k in range(NCORES):
        out += res_b[k]["partial"]
    out += bl32[None, :]
    return out
